# revision 10
# baseline (speedup 1.0000x reference)
"""Trainium2 Bass kernel for windowed multi-head attention (nn_AttentionWindow).

Reference computation (B=64, N=197, DIM=768, H=12, HD=64):
    qkv  = x @ qkv_w.T + [q_bias, 0, v_bias]
    q, k, v = split(qkv);  q *= HD**-0.5
    attn = softmax(q @ k.T + rpb_table[rel_index])
    out  = (attn @ v) @ proj_w.T + proj_b

Sharding: data-parallel over batch across 8 NeuronCores (8 batches/core).

Per-core design (bf16 matmuls on TensorE, fp32 PSUM accumulation):
  - x pre-transposed on host to xT [768, 1576] bf16 (feature-major),
    resident in SBUF. q,k computed feature-major into one resident
    [128, 12, 1576] tile, tiled over 512-token slices (batch-agnostic).
  - v: token-major per batch ([tokens, channels], 128+69 row chunks) so
    the attention contraction has tokens on partitions; stored bf16.
  - Scores transposed: S^T[j,i] = k_h[:,j]^T q_h, per (head-pair,
    batch) item. The two heads of a pair use opposite 64-row halves of
    the PE array (row-packing -> concurrent matmuls). Softmax WITHOUT
    max subtraction (scores are O(1): q pre-scaled by 1/8): exp on
    ScalarE (both heads' score chunks merged into one strided
    activation) -> P^T bf16, then bias multiply against the precomputed
    exp(bias) table (j1 chunk on VectorE, j2 chunk on GpSimd):
    softmax(S+B) = expS*expB / sums.
  - O^T[d,i] = sum_j v[j,d] P^T[j,i]: head pair col-packed via
    tile_position (0,0)/(0,64); softmax denominators via ones[128,64]
    matmuls, ALSO col-packed by head into the same PSUM tile as O^T
    (cols 256:453) -> one reciprocal_approx_fast + one merged
    normalization multiply per item on VectorE.
  - Schedule is batch-major: qk chunks first, then per batch
    v[b] -> 6 attention items -> proj[b]; the v/proj matmul streams
    fill TensorE during softmax latencies and output DMA overlaps.
  - PSUM->SBUF copies (qk, v, proj) run on GpSimd, freeing ScalarE for
    the exps and VectorE for recip/normalize.
"""
import sys
import functools

sys.path.insert(0, "/opt/trn_rl_repo")

import numpy as np
import ml_dtypes

import concourse.bass as bass  # noqa: E402
import concourse.bacc as bacc  # noqa: E402
import concourse.mybir as mybir  # noqa: E402
from concourse.tile import TileContext  # noqa: E402
from concourse.bass_utils import run_bass_kernel_spmd  # noqa: E402

F32 = mybir.dt.float32
BF16 = mybir.dt.bfloat16

NCORES = 8
B, NT, DIM = 64, 197, 768
H, HD = 12, 64
SCALE = HD ** -0.5  # 0.125, exact power of two -> folded into q weights
BPC = B // NCORES   # 8 batches per core
TOK = BPC * NT      # 1576 tokens per core
KC = DIM // 128     # 6
NT2 = NT - 128      # 69 (second token chunk)
SKEW = 2            # attention software-pipeline depth (items)
# 512-token slices for the token-parallel qk matmul
SLICES = [(s * 512, min(TOK, (s + 1) * 512)) for s in range((TOK + 511) // 512)]


def build(qkv_bias_nonzero: bool, proj_bias_nonzero: bool):
    nc = bacc.Bacc("TRN2", target_bir_lowering=False, debug=False)

    xt = nc.dram_tensor("xt", [DIM, TOK], BF16, kind="ExternalInput")
    qkw = nc.dram_tensor("qkw", [DIM, 2 * DIM], BF16, kind="ExternalInput")
    vw = nc.dram_tensor("vw", [DIM, DIM], BF16, kind="ExternalInput")
    pw = nc.dram_tensor("pw", [DIM, DIM], BF16, kind="ExternalInput")
    eb1 = nc.dram_tensor("eb1", [128, H * NT], BF16, kind="ExternalInput")
    eb2 = nc.dram_tensor("eb2", [NT2, H * NT], BF16, kind="ExternalInput")
    out = nc.dram_tensor("out", [DIM, TOK], F32, kind="ExternalOutput")
    if qkv_bias_nonzero:
        qkb = nc.dram_tensor("qkb", [1, 2 * DIM], BF16, kind="ExternalInput")
        vb = nc.dram_tensor("vb", [1, DIM], BF16, kind="ExternalInput")
    if proj_bias_nonzero:
        pb = nc.dram_tensor("pb", [1, DIM], BF16, kind="ExternalInput")

    with TileContext(nc) as tc:
        with (
            tc.tile_pool(name="const", bufs=1) as constp,
            tc.tile_pool(name="vp", bufs=6) as vp,
            tc.tile_pool(name="pp", bufs=2 * (SKEW + 2)) as pp,
            tc.tile_pool(name="rcp", bufs=3) as rcp,
            tc.tile_pool(name="obp", bufs=4) as obp,
            tc.tile_pool(name="mm", bufs=2, space="PSUM") as mm,
            tc.tile_pool(name="ots", bufs=2, space="PSUM") as ots,
            tc.tile_pool(name="sta", bufs=2, space="PSUM") as sta,
        ):
            # ---- resident constants & activations ----
            # DMA order matters: qk needs xb+qkw first; vw next (v of
            # batch 0 runs right after qk); eb tables before the first
            # bias-multiply; pw last (proj is latest consumer).
            xb_s = constp.tile([128, KC, TOK], BF16, name="xb_s")
            qkw_s = constp.tile([128, KC, 2 * DIM], BF16, name="qkw_s")
            vw_s = constp.tile([128, KC, DIM], BF16, name="vw_s")
            pw_s = constp.tile([128, KC, DIM], BF16, name="pw_s")
            # descriptor pushes cost ~0.6us each on the issuing engine's
            # queue -- spread the input loads over four engines so the
            # transfers all start within ~1us of kernel entry
            for kc in range(KC):
                nc.sync.dma_start(xb_s[:, kc, :], xt[kc * 128:(kc + 1) * 128, :])
            for kc in range(KC):
                nc.gpsimd.dma_start(qkw_s[:, kc, :],
                                    qkw[kc * 128:(kc + 1) * 128, :])
            for kc in range(KC):
                nc.scalar.dma_start(vw_s[:, kc, :],
                                    vw[kc * 128:(kc + 1) * 128, :])
            eb1_s = constp.tile([128, H * NT], BF16, name="eb1_s")
            eb2_s = constp.tile([NT2, H * NT], BF16, name="eb2_s")
            nc.scalar.dma_start(eb1_s[:, :], eb1[:, :])
            nc.scalar.dma_start(eb2_s[:, :], eb2[:, :])
            for kc in range(KC):
                nc.gpsimd.dma_start(pw_s[:, kc, :],
                                    pw[kc * 128:(kc + 1) * 128, :])
            # ones [128, 64] lhsT for the col-packed denominator matmuls
            ones64 = constp.tile([128, 64], BF16, name="ones64")
            nc.gpsimd.memset(ones64[:, :], 1.0)
            # big resident activations: q,k and proj-rhs (bf16)
            qk_s = constp.tile([128, 2 * KC, TOK], BF16, name="qk_s")
            op_s = constp.tile([128, KC, TOK], BF16, name="op_s")
            if qkv_bias_nonzero:
                qkb_s = constp.tile([1, 2 * DIM], BF16, name="qkb_s")
                vb_s = constp.tile([1, DIM], BF16, name="vb_s")
                nc.sync.dma_start(qkb_s[:, :], qkb[:, :])
                nc.sync.dma_start(vb_s[:, :], vb[:, :])
            if proj_bias_nonzero:
                pb_s = constp.tile([1, DIM], BF16, name="pb_s")
                nc.sync.dma_start(pb_s[:, :], pb[:, :])
            if qkv_bias_nonzero or proj_bias_nonzero:
                ones_bfr = constp.tile([1, 512], BF16, name="ones_bfr")
                nc.gpsimd.memset(ones_bfr[:, :], 1.0)

            # ---- q,k feature-major: 12 channel-chunks x token slices ----
            for c in range(2 * KC):
                for t0, t1 in SLICES:
                    acc = mm.tile([128, 512], F32, name="acc_qk", tag="mm")
                    w = t1 - t0
                    for kc in range(KC):
                        nc.tensor.matmul(
                            acc[:, 0:w],
                            qkw_s[:, kc, c * 128:(c + 1) * 128],
                            xb_s[:, kc, t0:t1],
                            start=(kc == 0),
                            stop=(kc == KC - 1) and not qkv_bias_nonzero,
                        )
                    if qkv_bias_nonzero:
                        nc.tensor.matmul(
                            acc[:, 0:w],
                            qkb_s[0:1, c * 128:(c + 1) * 128],
                            ones_bfr[0:1, 0:w],
                            start=False, stop=True,
                        )
                    # PSUM->SBUF: alternate scalar/vector (both idle here)
                    if (c * len(SLICES) + (t0 // 512)) % 2 == 0:
                        nc.scalar.copy(qk_s[:, c, t0:t1], acc[:, 0:w])
                    else:
                        nc.vector.tensor_copy(qk_s[:, c, t0:t1], acc[:, 0:w])

            # ---- v token-major for one batch: [(128|69) tok, 768 ch] ----
            vt = [[None, None] for _ in range(BPC)]

            def emit_v(b):
                for tch in range(2):
                    toff = b * NT + tch * 128
                    tlen = 128 if tch == 0 else NT2
                    t = vp.tile([128, DIM], BF16, name="v_t", tag="v")
                    for half in range(2):
                        n0, n1 = half * 384, (half + 1) * 384
                        acc = mm.tile([128, 384], F32, name="acc_v", tag="mm")
                        for kc in range(KC):
                            nc.tensor.matmul(
                                acc[0:tlen, :],
                                xb_s[:, kc, toff:toff + tlen],
                                vw_s[:, kc, n0:n1],
                                start=(kc == 0),
                                stop=(kc == KC - 1) and not qkv_bias_nonzero,
                            )
                        if qkv_bias_nonzero:
                            nc.tensor.matmul(
                                acc[0:tlen, :],
                                ones_bfr[0:1, 0:tlen],
                                vb_s[0:1, n0:n1],
                                start=False, stop=True,
                            )
                        if half == 0:
                            nc.scalar.copy(t[0:tlen, n0:n1], acc[0:tlen, :])
                        else:
                            nc.vector.tensor_copy(t[0:tlen, n0:n1],
                                                  acc[0:tlen, :])
                    vt[b][tch] = t

            # ---- attention stages (per (batch, head-pair) item) ----
            def stage_a(b, hp):
                """Scores for heads 2hp,2hp+1 (array row-packed), merged
                exp + bias-mult (j1 on vector, j2 on gpsimd) -> P^T."""
                h0 = 2 * hp
                st = sta.tile([128, 1024], F32, name="st", tag="sta")
                q0 = qk_s[0:64, hp, b * NT:(b + 1) * NT]
                q1 = qk_s[64:128, hp, b * NT:(b + 1) * NT]
                # j1 chunks (K rows 0:64 then 64:128 -> packed), then j2
                nc.tensor.matmul(st[:, 0:NT],
                                 qk_s[0:64, KC + hp, b * NT:b * NT + 128],
                                 q0, start=True, stop=True)
                nc.tensor.matmul(st[:, 512:512 + NT],
                                 qk_s[64:128, KC + hp, b * NT:b * NT + 128],
                                 q1, start=True, stop=True)
                nc.tensor.matmul(st[0:NT2, 256:256 + NT],
                                 qk_s[0:64, KC + hp, b * NT + 128:(b + 1) * NT],
                                 q0, start=True, stop=True)
                nc.tensor.matmul(st[0:NT2, 768:768 + NT],
                                 qk_s[64:128, KC + hp, b * NT + 128:(b + 1) * NT],
                                 q1, start=True, stop=True)
                # merged exp over both heads (strided free AP), bf16 out
                pj1 = pp.tile([128, 2, NT], BF16, name="pj1", tag="p")
                nc.scalar.activation(
                    pj1[:, :, :], st[:, 0:1024].rearrange("p (h x) -> p h x", h=2)[:, :, 0:NT],
                    mybir.ActivationFunctionType.Exp)
                pj2 = pp.tile([128, 2, NT], BF16, name="pj2", tag="p")
                nc.scalar.activation(
                    pj2[0:NT2, :, :],
                    st[0:NT2, 0:1024].rearrange("p (h x) -> p h x", h=2)[:, :, 256:256 + NT],
                    mybir.ActivationFunctionType.Exp)
                # bias multiply (heads adjacent in eb tables)
                nc.vector.tensor_mul(
                    pj1[:, :, :].rearrange("p h x -> p (h x)"),
                    pj1[:, :, :].rearrange("p h x -> p (h x)"),
                    eb1_s[:, h0 * NT:(h0 + 2) * NT])
                nc.gpsimd.tensor_mul(
                    pj2[0:NT2, :, :].rearrange("p h x -> p (h x)"),
                    pj2[0:NT2, :, :].rearrange("p h x -> p (h x)"),
                    eb2_s[:, h0 * NT:(h0 + 2) * NT])
                return pj1, pj2

            def stage_b(b, hp, pj1, pj2):
                """O^T and denominators, both head-pair col-packed into
                one PSUM tile; one recip + one merged normalize."""
                h0, h1 = 2 * hp, 2 * hp + 1
                ot = ots.tile([128, 512], F32, name="ot", tag="ot")
                nc.tensor.matmul(
                    ot[0:64, 0:NT],
                    vt[b][0][:, h0 * HD:(h0 + 1) * HD],
                    pj1[:, 0, :], start=True, stop=False)
                nc.tensor.matmul(
                    ot[64:128, 0:NT],
                    vt[b][0][:, h1 * HD:(h1 + 1) * HD],
                    pj1[:, 1, :], start=True, stop=False,
                    tile_position=(0, 64))
                nc.tensor.matmul(
                    ot[0:64, 0:NT],
                    vt[b][1][0:NT2, h0 * HD:(h0 + 1) * HD],
                    pj2[0:NT2, 0, :], start=False, stop=True)
                nc.tensor.matmul(
                    ot[64:128, 0:NT],
                    vt[b][1][0:NT2, h1 * HD:(h1 + 1) * HD],
                    pj2[0:NT2, 1, :], start=False, stop=True,
                    tile_position=(0, 64))
                # denominators: ones[128,64] lhsT, heads col-packed into
                # cols 256:256+NT of the same tile (partitions 0:64 = h0,
                # 64:128 = h1)
                nc.tensor.matmul(
                    ot[0:64, 256:256 + NT], ones64[:, :],
                    pj1[:, 0, :], start=True, stop=False)
                nc.tensor.matmul(
                    ot[64:128, 256:256 + NT], ones64[:, :],
                    pj1[:, 1, :], start=True, stop=False,
                    tile_position=(0, 64))
                nc.tensor.matmul(
                    ot[0:64, 256:256 + NT], ones64[0:NT2, :],
                    pj2[0:NT2, 0, :], start=False, stop=True)
                nc.tensor.matmul(
                    ot[64:128, 256:256 + NT], ones64[0:NT2, :],
                    pj2[0:NT2, 1, :], start=False, stop=True,
                    tile_position=(0, 64))
                rc = rcp.tile([128, NT], F32, name="rc", tag="rc")
                nc.vector.reciprocal_approx_fast(
                    out=rc[:, :], in_=ot[:, 256:256 + NT])
                nc.vector.tensor_mul(
                    op_s[:, hp, b * NT:(b + 1) * NT],
                    ot[:, 0:NT], rc[:, :])

            # ---- projection over a token range (1-2 batches) ----
            def emit_proj(t0, t1):
                w = t1 - t0
                for c in range(KC):
                    acc = mm.tile([128, 512], F32, name="acc_p", tag="mm")
                    for kp in range(KC):
                        nc.tensor.matmul(
                            acc[:, 0:w],
                            pw_s[:, kp, c * 128:(c + 1) * 128],
                            op_s[:, kp, t0:t1],
                            start=(kp == 0),
                            stop=(kp == KC - 1) and not proj_bias_nonzero,
                        )
                    if proj_bias_nonzero:
                        nc.tensor.matmul(
                            acc[:, 0:w],
                            pb_s[0:1, c * 128:(c + 1) * 128],
                            ones_bfr[0:1, 0:w],
                            start=False, stop=True,
                        )
                    obt = obp.tile([128, 2 * NT], F32, name="obt", tag="ob")
                    if c % 2 == 0:
                        nc.vector.tensor_copy(obt[:, 0:w], acc[:, 0:w])
                    else:
                        nc.scalar.copy(obt[:, 0:w], acc[:, 0:w])
                    nc.sync.dma_start(out[c * 128:(c + 1) * 128, t0:t1],
                                      obt[:, 0:w])

            # ---- batch-major schedule: v[b] -> items -> proj[b] ----
            items = [(b, hp) for b in range(BPC) for hp in range(KC)]
            pend = {}

            def run_stage_b(i):
                b, hp = items[i]
                stage_b(b, hp, *pend.pop(i))
                if hp == KC - 1:
                    # batches 0-5 project in pairs (394-wide rhs halves
                    # per-matmul overhead); 6 and 7 go solo so the last
                    # batch's projection isn't delayed into the tail
                    if b in (1, 3, 5):
                        emit_proj((b - 1) * NT, (b + 1) * NT)
                    elif b >= 6:
                        emit_proj(b * NT, (b + 1) * NT)

            for i, (b, hp) in enumerate(items):
                if hp == 0:
                    emit_v(b)
                pend[i] = stage_a(b, hp)
                if i >= SKEW:
                    run_stage_b(i - SKEW)
            for i in range(len(items) - SKEW, len(items)):
                run_stage_b(i)

    nc.compile()
    return nc


@functools.lru_cache(maxsize=4)
def _built(qkv_bias_nonzero: bool, proj_bias_nonzero: bool):
    return build(qkv_bias_nonzero, proj_bias_nonzero)


def prepare_inputs(x, qkv_w, q_bias, v_bias, rpb_table, proj_w, proj_b, rel_index):
    """Host-side prep: shard + transpose + fold scale + gather bias table."""
    x = np.asarray(x, dtype=np.float32)
    qkv_w = np.asarray(qkv_w, dtype=np.float32)
    q_bias = np.asarray(q_bias, dtype=np.float32)
    v_bias = np.asarray(v_bias, dtype=np.float32)
    rpb_table = np.asarray(rpb_table, dtype=np.float32)
    proj_w = np.asarray(proj_w, dtype=np.float32)
    proj_b = np.asarray(proj_b, dtype=np.float32)
    rel_index = np.asarray(rel_index)

    qw = qkv_w[0:DIM] * np.float32(SCALE)   # exact: SCALE is a power of two
    qkw_h = np.ascontiguousarray(
        np.concatenate([qw, qkv_w[DIM:2 * DIM]], axis=0).T).astype(
        ml_dtypes.bfloat16)                                      # [768, 1536]
    vw_h = np.ascontiguousarray(qkv_w[2 * DIM:3 * DIM].T).astype(
        ml_dtypes.bfloat16)                                      # [768, 768]
    pw_h = np.ascontiguousarray(proj_w.T).astype(ml_dtypes.bfloat16)

    # bias[i, j, h] -> exp -> ebT[h, j, i]
    bias = rpb_table[rel_index]                                  # (197,197,12)
    ebT = np.exp(bias.astype(np.float32)).transpose(2, 1, 0)     # (12, j, i)
    eb1_h = np.ascontiguousarray(
        ebT[:, 0:128, :].transpose(1, 0, 2).reshape(128, H * NT)
    ).astype(ml_dtypes.bfloat16)
    eb2_h = np.ascontiguousarray(
        ebT[:, 128:NT, :].transpose(1, 0, 2).reshape(NT2, H * NT)
    ).astype(ml_dtypes.bfloat16)

    qkv_bias_nonzero = bool(q_bias.any() or v_bias.any())
    proj_bias_nonzero = bool(proj_b.any())

    in_maps = []
    for i in range(NCORES):
        xs = x[i * BPC:(i + 1) * BPC].reshape(TOK, DIM)
        m = {
            "xt": np.ascontiguousarray(xs.T).astype(ml_dtypes.bfloat16),
            "qkw": qkw_h, "vw": vw_h, "pw": pw_h,
            "eb1": eb1_h, "eb2": eb2_h,
        }
        if qkv_bias_nonzero:
            m["qkb"] = np.ascontiguousarray(
                np.concatenate([q_bias * np.float32(SCALE),
                                np.zeros_like(q_bias)])[None, :],
                dtype=np.float32).astype(ml_dtypes.bfloat16)
            m["vb"] = np.ascontiguousarray(
                v_bias[None, :]).astype(ml_dtypes.bfloat16)
        if proj_bias_nonzero:
            m["pb"] = np.ascontiguousarray(
                proj_b[None, :], dtype=np.float32).astype(ml_dtypes.bfloat16)
        in_maps.append(m)
    return in_maps, qkv_bias_nonzero, proj_bias_nonzero


def kernel(x, qkv_w, q_bias, v_bias, rpb_table, proj_w, proj_b, rel_index):
    in_maps, qb_nz, pb_nz = prepare_inputs(
        x, qkv_w, q_bias, v_bias, rpb_table, proj_w, proj_b, rel_index)
    nc = _built(qb_nz, pb_nz)
    res = run_bass_kernel_spmd(nc, in_maps, core_ids=list(range(NCORES)))
    outs = []
    for i in range(NCORES):
        ofm = res.results[i]["out"]                  # [768, 1576]
        outs.append(ofm.T.reshape(BPC, NT, DIM))
    return np.concatenate(outs, axis=0).astype(np.float32)


# revision 11
# speedup vs baseline: 1.0525x; 1.0525x over previous
"""Trainium2 Bass kernel for windowed multi-head attention (nn_AttentionWindow).

Reference computation (B=64, N=197, DIM=768, H=12, HD=64):
    qkv  = x @ qkv_w.T + [q_bias, 0, v_bias]
    q, k, v = split(qkv);  q *= HD**-0.5
    attn = softmax(q @ k.T + rpb_table[rel_index])
    out  = (attn @ v) @ proj_w.T + proj_b

Sharding: data-parallel over batch across 8 NeuronCores (8 batches/core).

Per-core design (bf16 matmuls on TensorE, fp32 PSUM accumulation):
  - x pre-transposed on host to xT [768, 1576] bf16 (feature-major),
    resident in SBUF. q,k computed feature-major into one resident
    [128, 12, 1576] tile, tiled over 512-token slices (batch-agnostic).
  - v: token-major per batch ([tokens, channels], 128+69 row chunks) so
    the attention contraction has tokens on partitions; stored bf16.
  - Scores transposed: S^T[j,i] = k_h[:,j]^T q_h, per (head-pair,
    batch) item. The two heads of a pair use opposite 64-row halves of
    the PE array (row-packing -> concurrent matmuls). Softmax WITHOUT
    max subtraction (scores are O(1): q pre-scaled by 1/8): exp on
    ScalarE (both heads' score chunks merged into one strided
    activation) -> P^T bf16, then bias multiply against the precomputed
    exp(bias) table (j1 chunk on VectorE, j2 chunk on GpSimd):
    softmax(S+B) = expS*expB / sums.
  - O^T[d,i] = sum_j v[j,d] P^T[j,i]: head pair col-packed via
    tile_position (0,0)/(0,64); softmax denominators via ones[128,64]
    matmuls, ALSO col-packed by head into the same PSUM tile as O^T
    (cols 256:453) -> one reciprocal_approx_fast + one merged
    normalization multiply per item on VectorE.
  - Schedule is batch-major: qk chunks first, then per batch
    v[b] -> 6 attention items -> proj[b]; the v/proj matmul streams
    fill TensorE during softmax latencies and output DMA overlaps.
  - PSUM->SBUF copies (qk, v, proj) run on GpSimd, freeing ScalarE for
    the exps and VectorE for recip/normalize.
"""
import sys
import functools

sys.path.insert(0, "/opt/trn_rl_repo")

import numpy as np
import ml_dtypes

import concourse.bass as bass  # noqa: E402
import concourse.bacc as bacc  # noqa: E402
import concourse.mybir as mybir  # noqa: E402
from concourse.tile import TileContext  # noqa: E402
from concourse.bass_utils import run_bass_kernel_spmd  # noqa: E402

F32 = mybir.dt.float32
BF16 = mybir.dt.bfloat16

NCORES = 8
B, NT, DIM = 64, 197, 768
H, HD = 12, 64
SCALE = HD ** -0.5  # 0.125, exact power of two -> folded into q weights
BPC = B // NCORES   # 8 batches per core
TOK = BPC * NT      # 1576 tokens per core
KC = DIM // 128     # 6
NT2 = NT - 128      # 69 (second token chunk)
SKEW = 2            # attention software-pipeline depth (items)
# 512-token slices for the token-parallel qk matmul
SLICES = [(s * 512, min(TOK, (s + 1) * 512)) for s in range((TOK + 511) // 512)]


def build(qkv_bias_nonzero: bool, proj_bias_nonzero: bool):
    nc = bacc.Bacc("TRN2", target_bir_lowering=False, debug=False)

    xt = nc.dram_tensor("xt", [DIM, TOK], BF16, kind="ExternalInput")
    qkw = nc.dram_tensor("qkw", [DIM, 2 * DIM], BF16, kind="ExternalInput")
    vw = nc.dram_tensor("vw", [DIM, DIM], BF16, kind="ExternalInput")
    pw = nc.dram_tensor("pw", [DIM, DIM], BF16, kind="ExternalInput")
    eb1 = nc.dram_tensor("eb1", [128, H * NT], BF16, kind="ExternalInput")
    eb2 = nc.dram_tensor("eb2", [NT2, H * NT], BF16, kind="ExternalInput")
    out = nc.dram_tensor("out", [DIM, TOK], F32, kind="ExternalOutput")
    if qkv_bias_nonzero:
        qkb = nc.dram_tensor("qkb", [1, 2 * DIM], BF16, kind="ExternalInput")
        vb = nc.dram_tensor("vb", [1, DIM], BF16, kind="ExternalInput")
    if proj_bias_nonzero:
        pb = nc.dram_tensor("pb", [1, DIM], BF16, kind="ExternalInput")

    with TileContext(nc) as tc:
        with (
            tc.tile_pool(name="const", bufs=1) as constp,
            tc.tile_pool(name="vp", bufs=6) as vp,
            tc.tile_pool(name="pp", bufs=2 * (SKEW + 2)) as pp,
            tc.tile_pool(name="rcp", bufs=3) as rcp,
            tc.tile_pool(name="obp", bufs=4) as obp,
            tc.tile_pool(name="mm", bufs=2, space="PSUM") as mm,
            tc.tile_pool(name="ots", bufs=2, space="PSUM") as ots,
            tc.tile_pool(name="sta", bufs=2, space="PSUM") as sta,
        ):
            # ---- resident constants & activations ----
            # DMA order matters: qk needs xb+qkw first; vw next (v of
            # batch 0 runs right after qk); eb tables before the first
            # bias-multiply; pw last (proj is latest consumer).
            xb_s = constp.tile([128, KC, TOK], BF16, name="xb_s")
            qkw_s = constp.tile([128, KC, 2 * DIM], BF16, name="qkw_s")
            vw_s = constp.tile([128, KC, DIM], BF16, name="vw_s")
            pw_s = constp.tile([128, KC, DIM], BF16, name="pw_s")
            # all input DMAs on the sync hardware-DGE queue (engine-issued
            # DMAs fall back to slow software-dynamic queues); xb/qkw
            # chunks interleaved so qk compute starts feeding ASAP
            for kc in range(KC):
                nc.sync.dma_start(xb_s[:, kc, :], xt[kc * 128:(kc + 1) * 128, :])
                nc.sync.dma_start(qkw_s[:, kc, :],
                                  qkw[kc * 128:(kc + 1) * 128, :])
            for kc in range(KC):
                nc.sync.dma_start(vw_s[:, kc, :], vw[kc * 128:(kc + 1) * 128, :])
            eb1_s = constp.tile([128, H * NT], BF16, name="eb1_s")
            eb2_s = constp.tile([NT2, H * NT], BF16, name="eb2_s")
            nc.sync.dma_start(eb1_s[:, :], eb1[:, :])
            nc.sync.dma_start(eb2_s[:, :], eb2[:, :])
            for kc in range(KC):
                nc.sync.dma_start(pw_s[:, kc, :], pw[kc * 128:(kc + 1) * 128, :])
            # ones [128, 64] lhsT for the col-packed denominator matmuls
            ones64 = constp.tile([128, 64], BF16, name="ones64")
            nc.gpsimd.memset(ones64[:, :], 1.0)
            # big resident activations: q,k and proj-rhs (bf16)
            qk_s = constp.tile([128, 2 * KC, TOK], BF16, name="qk_s")
            op_s = constp.tile([128, KC, TOK], BF16, name="op_s")
            if qkv_bias_nonzero:
                qkb_s = constp.tile([1, 2 * DIM], BF16, name="qkb_s")
                vb_s = constp.tile([1, DIM], BF16, name="vb_s")
                nc.sync.dma_start(qkb_s[:, :], qkb[:, :])
                nc.sync.dma_start(vb_s[:, :], vb[:, :])
            if proj_bias_nonzero:
                pb_s = constp.tile([1, DIM], BF16, name="pb_s")
                nc.sync.dma_start(pb_s[:, :], pb[:, :])
            if qkv_bias_nonzero or proj_bias_nonzero:
                ones_bfr = constp.tile([1, 512], BF16, name="ones_bfr")
                nc.gpsimd.memset(ones_bfr[:, :], 1.0)

            # ---- q,k feature-major: 12 channel-chunks x token slices ----
            for c in range(2 * KC):
                for t0, t1 in SLICES:
                    acc = mm.tile([128, 512], F32, name="acc_qk", tag="mm")
                    w = t1 - t0
                    for kc in range(KC):
                        nc.tensor.matmul(
                            acc[:, 0:w],
                            qkw_s[:, kc, c * 128:(c + 1) * 128],
                            xb_s[:, kc, t0:t1],
                            start=(kc == 0),
                            stop=(kc == KC - 1) and not qkv_bias_nonzero,
                        )
                    if qkv_bias_nonzero:
                        nc.tensor.matmul(
                            acc[:, 0:w],
                            qkb_s[0:1, c * 128:(c + 1) * 128],
                            ones_bfr[0:1, 0:w],
                            start=False, stop=True,
                        )
                    # PSUM->SBUF: alternate scalar/vector (both idle here)
                    if (c * len(SLICES) + (t0 // 512)) % 2 == 0:
                        nc.scalar.copy(qk_s[:, c, t0:t1], acc[:, 0:w])
                    else:
                        nc.vector.tensor_copy(qk_s[:, c, t0:t1], acc[:, 0:w])

            # ---- v token-major for one batch: [(128|69) tok, 768 ch] ----
            vt = [[None, None] for _ in range(BPC)]

            def emit_v(b):
                for tch in range(2):
                    toff = b * NT + tch * 128
                    tlen = 128 if tch == 0 else NT2
                    t = vp.tile([128, DIM], BF16, name="v_t", tag="v")
                    for half in range(2):
                        n0, n1 = half * 384, (half + 1) * 384
                        acc = mm.tile([128, 384], F32, name="acc_v", tag="mm")
                        for kc in range(KC):
                            nc.tensor.matmul(
                                acc[0:tlen, :],
                                xb_s[:, kc, toff:toff + tlen],
                                vw_s[:, kc, n0:n1],
                                start=(kc == 0),
                                stop=(kc == KC - 1) and not qkv_bias_nonzero,
                            )
                        if qkv_bias_nonzero:
                            nc.tensor.matmul(
                                acc[0:tlen, :],
                                ones_bfr[0:1, 0:tlen],
                                vb_s[0:1, n0:n1],
                                start=False, stop=True,
                            )
                        if half == 0:
                            nc.scalar.copy(t[0:tlen, n0:n1], acc[0:tlen, :])
                        else:
                            nc.vector.tensor_copy(t[0:tlen, n0:n1],
                                                  acc[0:tlen, :])
                    vt[b][tch] = t

            # ---- attention stages (per (batch, head-pair) item) ----
            def stage_a(b, hp):
                """Scores for heads 2hp,2hp+1 (array row-packed), merged
                exp + bias-mult (j1 on vector, j2 on gpsimd) -> P^T."""
                h0 = 2 * hp
                st = sta.tile([128, 1024], F32, name="st", tag="sta")
                q0 = qk_s[0:64, hp, b * NT:(b + 1) * NT]
                q1 = qk_s[64:128, hp, b * NT:(b + 1) * NT]
                # j1 chunks (K rows 0:64 then 64:128 -> packed), then j2
                nc.tensor.matmul(st[:, 0:NT],
                                 qk_s[0:64, KC + hp, b * NT:b * NT + 128],
                                 q0, start=True, stop=True)
                nc.tensor.matmul(st[:, 512:512 + NT],
                                 qk_s[64:128, KC + hp, b * NT:b * NT + 128],
                                 q1, start=True, stop=True)
                nc.tensor.matmul(st[0:NT2, 256:256 + NT],
                                 qk_s[0:64, KC + hp, b * NT + 128:(b + 1) * NT],
                                 q0, start=True, stop=True)
                nc.tensor.matmul(st[0:NT2, 768:768 + NT],
                                 qk_s[64:128, KC + hp, b * NT + 128:(b + 1) * NT],
                                 q1, start=True, stop=True)
                # merged exp over both heads (strided free AP), bf16 out
                pj1 = pp.tile([128, 2, NT], BF16, name="pj1", tag="p")
                nc.scalar.activation(
                    pj1[:, :, :], st[:, 0:1024].rearrange("p (h x) -> p h x", h=2)[:, :, 0:NT],
                    mybir.ActivationFunctionType.Exp)
                pj2 = pp.tile([128, 2, NT], BF16, name="pj2", tag="p")
                nc.scalar.activation(
                    pj2[0:NT2, :, :],
                    st[0:NT2, 0:1024].rearrange("p (h x) -> p h x", h=2)[:, :, 256:256 + NT],
                    mybir.ActivationFunctionType.Exp)
                # bias multiply (heads adjacent in eb tables)
                nc.vector.tensor_mul(
                    pj1[:, :, :].rearrange("p h x -> p (h x)"),
                    pj1[:, :, :].rearrange("p h x -> p (h x)"),
                    eb1_s[:, h0 * NT:(h0 + 2) * NT])
                nc.gpsimd.tensor_mul(
                    pj2[0:NT2, :, :].rearrange("p h x -> p (h x)"),
                    pj2[0:NT2, :, :].rearrange("p h x -> p (h x)"),
                    eb2_s[:, h0 * NT:(h0 + 2) * NT])
                return pj1, pj2

            def stage_b(b, hp, pj1, pj2):
                """O^T and denominators, both head-pair col-packed into
                one PSUM tile; one recip + one merged normalize."""
                h0, h1 = 2 * hp, 2 * hp + 1
                ot = ots.tile([128, 512], F32, name="ot", tag="ot")
                nc.tensor.matmul(
                    ot[0:64, 0:NT],
                    vt[b][0][:, h0 * HD:(h0 + 1) * HD],
                    pj1[:, 0, :], start=True, stop=False)
                nc.tensor.matmul(
                    ot[64:128, 0:NT],
                    vt[b][0][:, h1 * HD:(h1 + 1) * HD],
                    pj1[:, 1, :], start=True, stop=False,
                    tile_position=(0, 64))
                nc.tensor.matmul(
                    ot[0:64, 0:NT],
                    vt[b][1][0:NT2, h0 * HD:(h0 + 1) * HD],
                    pj2[0:NT2, 0, :], start=False, stop=True)
                nc.tensor.matmul(
                    ot[64:128, 0:NT],
                    vt[b][1][0:NT2, h1 * HD:(h1 + 1) * HD],
                    pj2[0:NT2, 1, :], start=False, stop=True,
                    tile_position=(0, 64))
                # denominators: ones[128,64] lhsT, heads col-packed into
                # cols 256:256+NT of the same tile (partitions 0:64 = h0,
                # 64:128 = h1)
                nc.tensor.matmul(
                    ot[0:64, 256:256 + NT], ones64[:, :],
                    pj1[:, 0, :], start=True, stop=False)
                nc.tensor.matmul(
                    ot[64:128, 256:256 + NT], ones64[:, :],
                    pj1[:, 1, :], start=True, stop=False,
                    tile_position=(0, 64))
                nc.tensor.matmul(
                    ot[0:64, 256:256 + NT], ones64[0:NT2, :],
                    pj2[0:NT2, 0, :], start=False, stop=True)
                nc.tensor.matmul(
                    ot[64:128, 256:256 + NT], ones64[0:NT2, :],
                    pj2[0:NT2, 1, :], start=False, stop=True,
                    tile_position=(0, 64))
                rc = rcp.tile([128, NT], F32, name="rc", tag="rc")
                nc.vector.reciprocal_approx_fast(
                    out=rc[:, :], in_=ot[:, 256:256 + NT])
                nc.vector.tensor_mul(
                    op_s[:, hp, b * NT:(b + 1) * NT],
                    ot[:, 0:NT], rc[:, :])

            # ---- projection over a token range (1-2 batches) ----
            def emit_proj(t0, t1):
                w = t1 - t0
                for c in range(KC):
                    acc = mm.tile([128, 512], F32, name="acc_p", tag="mm")
                    for kp in range(KC):
                        nc.tensor.matmul(
                            acc[:, 0:w],
                            pw_s[:, kp, c * 128:(c + 1) * 128],
                            op_s[:, kp, t0:t1],
                            start=(kp == 0),
                            stop=(kp == KC - 1) and not proj_bias_nonzero,
                        )
                    if proj_bias_nonzero:
                        nc.tensor.matmul(
                            acc[:, 0:w],
                            pb_s[0:1, c * 128:(c + 1) * 128],
                            ones_bfr[0:1, 0:w],
                            start=False, stop=True,
                        )
                    obt = obp.tile([128, 2 * NT], F32, name="obt", tag="ob")
                    if c % 2 == 0:
                        nc.vector.tensor_copy(obt[:, 0:w], acc[:, 0:w])
                    else:
                        nc.scalar.copy(obt[:, 0:w], acc[:, 0:w])
                    nc.sync.dma_start(out[c * 128:(c + 1) * 128, t0:t1],
                                      obt[:, 0:w])

            # ---- batch-major schedule: v[b] -> items -> proj[b] ----
            items = [(b, hp) for b in range(BPC) for hp in range(KC)]
            pend = {}

            def run_stage_b(i):
                b, hp = items[i]
                stage_b(b, hp, *pend.pop(i))
                if hp == KC - 1:
                    # batches 0-5 project in pairs (394-wide rhs halves
                    # per-matmul overhead); 6 and 7 go solo so the last
                    # batch's projection isn't delayed into the tail
                    if b in (1, 3, 5):
                        emit_proj((b - 1) * NT, (b + 1) * NT)
                    elif b >= 6:
                        emit_proj(b * NT, (b + 1) * NT)

            for i, (b, hp) in enumerate(items):
                if hp == 0:
                    emit_v(b)
                pend[i] = stage_a(b, hp)
                if i >= SKEW:
                    run_stage_b(i - SKEW)
            for i in range(len(items) - SKEW, len(items)):
                run_stage_b(i)

    nc.compile()
    return nc


@functools.lru_cache(maxsize=4)
def _built(qkv_bias_nonzero: bool, proj_bias_nonzero: bool):
    return build(qkv_bias_nonzero, proj_bias_nonzero)


def prepare_inputs(x, qkv_w, q_bias, v_bias, rpb_table, proj_w, proj_b, rel_index):
    """Host-side prep: shard + transpose + fold scale + gather bias table."""
    x = np.asarray(x, dtype=np.float32)
    qkv_w = np.asarray(qkv_w, dtype=np.float32)
    q_bias = np.asarray(q_bias, dtype=np.float32)
    v_bias = np.asarray(v_bias, dtype=np.float32)
    rpb_table = np.asarray(rpb_table, dtype=np.float32)
    proj_w = np.asarray(proj_w, dtype=np.float32)
    proj_b = np.asarray(proj_b, dtype=np.float32)
    rel_index = np.asarray(rel_index)

    qw = qkv_w[0:DIM] * np.float32(SCALE)   # exact: SCALE is a power of two
    qkw_h = np.ascontiguousarray(
        np.concatenate([qw, qkv_w[DIM:2 * DIM]], axis=0).T).astype(
        ml_dtypes.bfloat16)                                      # [768, 1536]
    vw_h = np.ascontiguousarray(qkv_w[2 * DIM:3 * DIM].T).astype(
        ml_dtypes.bfloat16)                                      # [768, 768]
    pw_h = np.ascontiguousarray(proj_w.T).astype(ml_dtypes.bfloat16)

    # bias[i, j, h] -> exp -> ebT[h, j, i]
    bias = rpb_table[rel_index]                                  # (197,197,12)
    ebT = np.exp(bias.astype(np.float32)).transpose(2, 1, 0)     # (12, j, i)
    eb1_h = np.ascontiguousarray(
        ebT[:, 0:128, :].transpose(1, 0, 2).reshape(128, H * NT)
    ).astype(ml_dtypes.bfloat16)
    eb2_h = np.ascontiguousarray(
        ebT[:, 128:NT, :].transpose(1, 0, 2).reshape(NT2, H * NT)
    ).astype(ml_dtypes.bfloat16)

    qkv_bias_nonzero = bool(q_bias.any() or v_bias.any())
    proj_bias_nonzero = bool(proj_b.any())

    in_maps = []
    for i in range(NCORES):
        xs = x[i * BPC:(i + 1) * BPC].reshape(TOK, DIM)
        m = {
            "xt": np.ascontiguousarray(xs.T).astype(ml_dtypes.bfloat16),
            "qkw": qkw_h, "vw": vw_h, "pw": pw_h,
            "eb1": eb1_h, "eb2": eb2_h,
        }
        if qkv_bias_nonzero:
            m["qkb"] = np.ascontiguousarray(
                np.concatenate([q_bias * np.float32(SCALE),
                                np.zeros_like(q_bias)])[None, :],
                dtype=np.float32).astype(ml_dtypes.bfloat16)
            m["vb"] = np.ascontiguousarray(
                v_bias[None, :]).astype(ml_dtypes.bfloat16)
        if proj_bias_nonzero:
            m["pb"] = np.ascontiguousarray(
                proj_b[None, :], dtype=np.float32).astype(ml_dtypes.bfloat16)
        in_maps.append(m)
    return in_maps, qkv_bias_nonzero, proj_bias_nonzero


def kernel(x, qkv_w, q_bias, v_bias, rpb_table, proj_w, proj_b, rel_index):
    in_maps, qb_nz, pb_nz = prepare_inputs(
        x, qkv_w, q_bias, v_bias, rpb_table, proj_w, proj_b, rel_index)
    nc = _built(qb_nz, pb_nz)
    res = run_bass_kernel_spmd(nc, in_maps, core_ids=list(range(NCORES)))
    outs = []
    for i in range(NCORES):
        ofm = res.results[i]["out"]                  # [768, 1576]
        outs.append(ofm.T.reshape(BPC, NT, DIM))
    return np.concatenate(outs, axis=0).astype(np.float32)


# revision 14
# speedup vs baseline: 1.1116x; 1.0561x over previous
"""Trainium2 Bass kernel for windowed multi-head attention (nn_AttentionWindow).

Reference computation (B=64, N=197, DIM=768, H=12, HD=64):
    qkv  = x @ qkv_w.T + [q_bias, 0, v_bias]
    q, k, v = split(qkv);  q *= HD**-0.5
    attn = softmax(q @ k.T + rpb_table[rel_index])
    out  = (attn @ v) @ proj_w.T + proj_b

Sharding: data-parallel over batch across 8 NeuronCores (8 batches/core).

Per-core design (bf16 matmuls on TensorE, fp32 PSUM accumulation):
  - x pre-transposed on host to xT [768, 1576] bf16 (feature-major),
    resident in SBUF. q,k computed feature-major into one resident
    [128, 12, 1576] tile, tiled over 512-token slices (batch-agnostic).
  - v: token-major per batch ([tokens, channels], 128+69 row chunks) so
    the attention contraction has tokens on partitions; stored bf16.
  - Scores transposed: S^T[j,i] = k_h[:,j]^T q_h, per (head-pair,
    batch) item. The two heads of a pair use opposite 64-row halves of
    the PE array (row-packing -> concurrent matmuls). Softmax WITHOUT
    max subtraction (scores are O(1): q pre-scaled by 1/8): exp on
    ScalarE (both heads' score chunks merged into one strided
    activation) -> P^T bf16, then bias multiply against the precomputed
    exp(bias) table (j1 chunk on VectorE, j2 chunk on GpSimd):
    softmax(S+B) = expS*expB / sums.
  - O^T[d,i] = sum_j v[j,d] P^T[j,i]: head pair col-packed via
    tile_position (0,0)/(0,64); softmax denominators via ones[128,64]
    matmuls, ALSO col-packed by head into the same PSUM tile as O^T
    (cols 256:453) -> one reciprocal_approx_fast + one merged
    normalization multiply per item on VectorE.
  - Schedule is batch-major: qk chunks first, then per batch
    v[b] -> 6 attention items -> proj[b]; the v/proj matmul streams
    fill TensorE during softmax latencies and output DMA overlaps.
  - PSUM->SBUF copies (qk, v, proj) run on GpSimd, freeing ScalarE for
    the exps and VectorE for recip/normalize.
"""
import sys
import functools

sys.path.insert(0, "/opt/trn_rl_repo")

import numpy as np
import ml_dtypes

import concourse.bass as bass  # noqa: E402
import concourse.bacc as bacc  # noqa: E402
import concourse.mybir as mybir  # noqa: E402
from concourse.tile import TileContext  # noqa: E402
from concourse.bass_utils import run_bass_kernel_spmd  # noqa: E402

F32 = mybir.dt.float32
BF16 = mybir.dt.bfloat16

NCORES = 8
B, NT, DIM = 64, 197, 768
H, HD = 12, 64
SCALE = HD ** -0.5  # 0.125, exact power of two -> folded into q weights
BPC = B // NCORES   # 8 batches per core
TOK = BPC * NT      # 1576 tokens per core
KC = DIM // 128     # 6
NT2 = NT - 128      # 69 (second token chunk)
SKEW = 2            # attention software-pipeline depth (items)
# 512-token slices for the token-parallel qk matmul
SLICES = [(s * 512, min(TOK, (s + 1) * 512)) for s in range((TOK + 511) // 512)]


def build(qkv_bias_nonzero: bool, proj_bias_nonzero: bool):
    nc = bacc.Bacc("TRN2", target_bir_lowering=False, debug=False)

    xt = nc.dram_tensor("xt", [DIM, TOK], BF16, kind="ExternalInput")
    qkw = nc.dram_tensor("qkw", [DIM, 2 * DIM], BF16, kind="ExternalInput")
    vw = nc.dram_tensor("vw", [DIM, DIM], BF16, kind="ExternalInput")
    pw = nc.dram_tensor("pw", [DIM, DIM], BF16, kind="ExternalInput")
    eb1 = nc.dram_tensor("eb1", [128, H * NT], BF16, kind="ExternalInput")
    eb2 = nc.dram_tensor("eb2", [NT2, H * NT], BF16, kind="ExternalInput")
    out = nc.dram_tensor("out", [DIM, TOK], F32, kind="ExternalOutput")
    if qkv_bias_nonzero:
        qkb = nc.dram_tensor("qkb", [1, 2 * DIM], BF16, kind="ExternalInput")
        vb = nc.dram_tensor("vb", [1, DIM], BF16, kind="ExternalInput")
    if proj_bias_nonzero:
        pb = nc.dram_tensor("pb", [1, DIM], BF16, kind="ExternalInput")

    with TileContext(nc) as tc:
        with (
            tc.tile_pool(name="const", bufs=1) as constp,
            tc.tile_pool(name="vp", bufs=6) as vp,
            tc.tile_pool(name="pp", bufs=2 * (SKEW + 2)) as pp,
            tc.tile_pool(name="rcp", bufs=3) as rcp,
            tc.tile_pool(name="obp", bufs=4) as obp,
            tc.tile_pool(name="mm", bufs=2, space="PSUM") as mm,
            tc.tile_pool(name="ots", bufs=2, space="PSUM") as ots,
            tc.tile_pool(name="sta", bufs=2, space="PSUM") as sta,
        ):
            # ---- resident constants & activations ----
            # DMA order matters: qk needs xb+qkw first; vw next (v of
            # batch 0 runs right after qk); eb tables before the first
            # bias-multiply; pw last (proj is latest consumer).
            xb_s = constp.tile([128, KC, TOK], BF16, name="xb_s")
            qkw_s = constp.tile([128, KC, 2 * DIM], BF16, name="qkw_s")
            vw_s = constp.tile([128, KC, DIM], BF16, name="vw_s")
            pw_s = constp.tile([128, KC, DIM], BF16, name="pw_s")
            # all input DMAs on the sync hardware-DGE queue (engine-issued
            # DMAs fall back to slow software-dynamic queues); xb/qkw
            # chunks interleaved so qk compute starts feeding ASAP
            for kc in range(KC):
                nc.sync.dma_start(xb_s[:, kc, :], xt[kc * 128:(kc + 1) * 128, :])
                nc.sync.dma_start(qkw_s[:, kc, :],
                                  qkw[kc * 128:(kc + 1) * 128, :])
            for kc in range(KC):
                nc.sync.dma_start(vw_s[:, kc, :], vw[kc * 128:(kc + 1) * 128, :])
            eb1_s = constp.tile([128, H * NT], BF16, name="eb1_s")
            eb2_s = constp.tile([NT2, H * NT], BF16, name="eb2_s")
            nc.sync.dma_start(eb1_s[:, :], eb1[:, :])
            nc.sync.dma_start(eb2_s[:, :], eb2[:, :])
            for kc in range(KC):
                nc.sync.dma_start(pw_s[:, kc, :], pw[kc * 128:(kc + 1) * 128, :])
            # ones [128, 64] lhsT for the col-packed denominator matmuls
            ones64 = constp.tile([128, 64], BF16, name="ones64")
            nc.gpsimd.memset(ones64[:, :], 1.0)
            # big resident activations: q,k and proj-rhs (bf16)
            qk_s = constp.tile([128, 2 * KC, TOK], BF16, name="qk_s")
            op_s = constp.tile([128, KC, TOK], BF16, name="op_s")
            if qkv_bias_nonzero:
                qkb_s = constp.tile([1, 2 * DIM], BF16, name="qkb_s")
                vb_s = constp.tile([1, DIM], BF16, name="vb_s")
                nc.sync.dma_start(qkb_s[:, :], qkb[:, :])
                nc.sync.dma_start(vb_s[:, :], vb[:, :])
            if proj_bias_nonzero:
                pb_s = constp.tile([1, DIM], BF16, name="pb_s")
                nc.sync.dma_start(pb_s[:, :], pb[:, :])
            if qkv_bias_nonzero or proj_bias_nonzero:
                ones_bfr = constp.tile([1, 512], BF16, name="ones_bfr")
                nc.gpsimd.memset(ones_bfr[:, :], 1.0)

            # ---- q,k feature-major: 12 channel-chunks x token slices ----
            for c in range(2 * KC):
                for t0, t1 in SLICES:
                    acc = mm.tile([128, 512], F32, name="acc_qk", tag="mm")
                    w = t1 - t0
                    for kc in range(KC):
                        nc.tensor.matmul(
                            acc[:, 0:w],
                            qkw_s[:, kc, c * 128:(c + 1) * 128],
                            xb_s[:, kc, t0:t1],
                            start=(kc == 0),
                            stop=(kc == KC - 1) and not qkv_bias_nonzero,
                        )
                    if qkv_bias_nonzero:
                        nc.tensor.matmul(
                            acc[:, 0:w],
                            qkb_s[0:1, c * 128:(c + 1) * 128],
                            ones_bfr[0:1, 0:w],
                            start=False, stop=True,
                        )
                    # PSUM->SBUF: alternate scalar/vector (both idle here)
                    if (c * len(SLICES) + (t0 // 512)) % 2 == 0:
                        nc.scalar.copy(qk_s[:, c, t0:t1], acc[:, 0:w])
                    else:
                        nc.vector.tensor_copy(qk_s[:, c, t0:t1], acc[:, 0:w])

            # ---- v token-major for one batch: [(128|69) tok, 768 ch] ----
            vt = [[None, None] for _ in range(BPC)]

            def emit_v(b):
                for tch in range(2):
                    toff = b * NT + tch * 128
                    tlen = 128 if tch == 0 else NT2
                    t = vp.tile([128, DIM], BF16, name="v_t", tag="v")
                    for half in range(2):
                        n0, n1 = half * 384, (half + 1) * 384
                        acc = mm.tile([128, 384], F32, name="acc_v", tag="mm")
                        for kc in range(KC):
                            nc.tensor.matmul(
                                acc[0:tlen, :],
                                xb_s[:, kc, toff:toff + tlen],
                                vw_s[:, kc, n0:n1],
                                start=(kc == 0),
                                stop=(kc == KC - 1) and not qkv_bias_nonzero,
                            )
                        if qkv_bias_nonzero:
                            nc.tensor.matmul(
                                acc[0:tlen, :],
                                ones_bfr[0:1, 0:tlen],
                                vb_s[0:1, n0:n1],
                                start=False, stop=True,
                            )
                        if half == 0:
                            nc.scalar.copy(t[0:tlen, n0:n1], acc[0:tlen, :])
                        else:
                            nc.vector.tensor_copy(t[0:tlen, n0:n1],
                                                  acc[0:tlen, :])
                    vt[b][tch] = t

            # ---- attention stages (per (batch, head-pair) item) ----
            def stage_a(b, hp):
                """Scores for heads 2hp,2hp+1 (array row-packed), merged
                exp + bias-mult (j1 on vector, j2 on gpsimd) -> P^T."""
                h0 = 2 * hp
                st = sta.tile([128, 1024], F32, name="st", tag="sta")
                q0 = qk_s[0:64, hp, b * NT:(b + 1) * NT]
                q1 = qk_s[64:128, hp, b * NT:(b + 1) * NT]
                # j1 chunks (K rows 0:64 then 64:128 -> packed), then j2
                nc.tensor.matmul(st[:, 0:NT],
                                 qk_s[0:64, KC + hp, b * NT:b * NT + 128],
                                 q0, start=True, stop=True)
                nc.tensor.matmul(st[:, 512:512 + NT],
                                 qk_s[64:128, KC + hp, b * NT:b * NT + 128],
                                 q1, start=True, stop=True)
                nc.tensor.matmul(st[0:NT2, 256:256 + NT],
                                 qk_s[0:64, KC + hp, b * NT + 128:(b + 1) * NT],
                                 q0, start=True, stop=True)
                nc.tensor.matmul(st[0:NT2, 768:768 + NT],
                                 qk_s[64:128, KC + hp, b * NT + 128:(b + 1) * NT],
                                 q1, start=True, stop=True)
                # merged exp over both heads (strided free AP), bf16 out
                pj1 = pp.tile([128, 2, NT], BF16, name="pj1", tag="p")
                nc.scalar.activation(
                    pj1[:, :, :], st[:, 0:1024].rearrange("p (h x) -> p h x", h=2)[:, :, 0:NT],
                    mybir.ActivationFunctionType.Exp)
                pj2 = pp.tile([128, 2, NT], BF16, name="pj2", tag="p")
                nc.scalar.activation(
                    pj2[0:NT2, :, :],
                    st[0:NT2, 0:1024].rearrange("p (h x) -> p h x", h=2)[:, :, 256:256 + NT],
                    mybir.ActivationFunctionType.Exp)
                # bias multiply (heads adjacent in eb tables)
                nc.vector.tensor_mul(
                    pj1[:, :, :].rearrange("p h x -> p (h x)"),
                    pj1[:, :, :].rearrange("p h x -> p (h x)"),
                    eb1_s[:, h0 * NT:(h0 + 2) * NT])
                nc.vector.tensor_mul(
                    pj2[0:NT2, :, :].rearrange("p h x -> p (h x)"),
                    pj2[0:NT2, :, :].rearrange("p h x -> p (h x)"),
                    eb2_s[:, h0 * NT:(h0 + 2) * NT])
                return pj1, pj2

            def stage_b(b, hp, pj1, pj2):
                """O^T and denominators, both head-pair col-packed into
                one PSUM tile; one recip + one merged normalize."""
                h0, h1 = 2 * hp, 2 * hp + 1
                ot = ots.tile([128, 512], F32, name="ot", tag="ot")
                nc.tensor.matmul(
                    ot[0:64, 0:NT],
                    vt[b][0][:, h0 * HD:(h0 + 1) * HD],
                    pj1[:, 0, :], start=True, stop=False)
                nc.tensor.matmul(
                    ot[64:128, 0:NT],
                    vt[b][0][:, h1 * HD:(h1 + 1) * HD],
                    pj1[:, 1, :], start=True, stop=False,
                    tile_position=(0, 64))
                nc.tensor.matmul(
                    ot[0:64, 0:NT],
                    vt[b][1][0:NT2, h0 * HD:(h0 + 1) * HD],
                    pj2[0:NT2, 0, :], start=False, stop=True)
                nc.tensor.matmul(
                    ot[64:128, 0:NT],
                    vt[b][1][0:NT2, h1 * HD:(h1 + 1) * HD],
                    pj2[0:NT2, 1, :], start=False, stop=True,
                    tile_position=(0, 64))
                # denominators: ones[128,64] lhsT, heads col-packed into
                # cols 256:256+NT of the same tile (partitions 0:64 = h0,
                # 64:128 = h1); accumulation groups in one PSUM bank must
                # stay sequential (interleaving open groups corrupts them)
                nc.tensor.matmul(
                    ot[0:64, 256:256 + NT], ones64[:, :],
                    pj1[:, 0, :], start=True, stop=False)
                nc.tensor.matmul(
                    ot[64:128, 256:256 + NT], ones64[:, :],
                    pj1[:, 1, :], start=True, stop=False,
                    tile_position=(0, 64))
                nc.tensor.matmul(
                    ot[0:64, 256:256 + NT], ones64[0:NT2, :],
                    pj2[0:NT2, 0, :], start=False, stop=True)
                nc.tensor.matmul(
                    ot[64:128, 256:256 + NT], ones64[0:NT2, :],
                    pj2[0:NT2, 1, :], start=False, stop=True,
                    tile_position=(0, 64))
                rc = rcp.tile([128, NT], F32, name="rc", tag="rc")
                nc.vector.reciprocal_approx_fast(
                    out=rc[:, :], in_=ot[:, 256:256 + NT])
                nc.vector.tensor_mul(
                    op_s[:, hp, b * NT:(b + 1) * NT],
                    ot[:, 0:NT], rc[:, :])

            # ---- projection over a token range (1-2 batches) ----
            def emit_proj(t0, t1):
                w = t1 - t0
                for c in range(KC):
                    acc = mm.tile([128, 512], F32, name="acc_p", tag="mm")
                    for kp in range(KC):
                        nc.tensor.matmul(
                            acc[:, 0:w],
                            pw_s[:, kp, c * 128:(c + 1) * 128],
                            op_s[:, kp, t0:t1],
                            start=(kp == 0),
                            stop=(kp == KC - 1) and not proj_bias_nonzero,
                        )
                    if proj_bias_nonzero:
                        nc.tensor.matmul(
                            acc[:, 0:w],
                            pb_s[0:1, c * 128:(c + 1) * 128],
                            ones_bfr[0:1, 0:w],
                            start=False, stop=True,
                        )
                    obt = obp.tile([128, 2 * NT], F32, name="obt", tag="ob")
                    if c % 2 == 0:
                        nc.vector.tensor_copy(obt[:, 0:w], acc[:, 0:w])
                    else:
                        nc.scalar.copy(obt[:, 0:w], acc[:, 0:w])
                    nc.sync.dma_start(out[c * 128:(c + 1) * 128, t0:t1],
                                      obt[:, 0:w])

            # ---- batch-major schedule: v[b] -> items -> proj[b] ----
            items = [(b, hp) for b in range(BPC) for hp in range(KC)]
            pend = {}

            def run_stage_b(i):
                b, hp = items[i]
                stage_b(b, hp, *pend.pop(i))
                if hp == KC - 1:
                    # batches 0-5 project in pairs (394-wide rhs halves
                    # per-matmul overhead); 6 and 7 go solo so the last
                    # batch's projection isn't delayed into the tail
                    if b in (1, 3, 5):
                        emit_proj((b - 1) * NT, (b + 1) * NT)
                    elif b >= 6:
                        emit_proj(b * NT, (b + 1) * NT)

            for i, (b, hp) in enumerate(items):
                if hp == 0:
                    emit_v(b)
                pend[i] = stage_a(b, hp)
                if i >= SKEW:
                    run_stage_b(i - SKEW)
            for i in range(len(items) - SKEW, len(items)):
                run_stage_b(i)

    nc.compile()
    return nc


@functools.lru_cache(maxsize=4)
def _built(qkv_bias_nonzero: bool, proj_bias_nonzero: bool):
    return build(qkv_bias_nonzero, proj_bias_nonzero)


def prepare_inputs(x, qkv_w, q_bias, v_bias, rpb_table, proj_w, proj_b, rel_index):
    """Host-side prep: shard + transpose + fold scale + gather bias table."""
    x = np.asarray(x, dtype=np.float32)
    qkv_w = np.asarray(qkv_w, dtype=np.float32)
    q_bias = np.asarray(q_bias, dtype=np.float32)
    v_bias = np.asarray(v_bias, dtype=np.float32)
    rpb_table = np.asarray(rpb_table, dtype=np.float32)
    proj_w = np.asarray(proj_w, dtype=np.float32)
    proj_b = np.asarray(proj_b, dtype=np.float32)
    rel_index = np.asarray(rel_index)

    qw = qkv_w[0:DIM] * np.float32(SCALE)   # exact: SCALE is a power of two
    qkw_h = np.ascontiguousarray(
        np.concatenate([qw, qkv_w[DIM:2 * DIM]], axis=0).T).astype(
        ml_dtypes.bfloat16)                                      # [768, 1536]
    vw_h = np.ascontiguousarray(qkv_w[2 * DIM:3 * DIM].T).astype(
        ml_dtypes.bfloat16)                                      # [768, 768]
    pw_h = np.ascontiguousarray(proj_w.T).astype(ml_dtypes.bfloat16)

    # bias[i, j, h] -> exp -> ebT[h, j, i]
    bias = rpb_table[rel_index]                                  # (197,197,12)
    ebT = np.exp(bias.astype(np.float32)).transpose(2, 1, 0)     # (12, j, i)
    eb1_h = np.ascontiguousarray(
        ebT[:, 0:128, :].transpose(1, 0, 2).reshape(128, H * NT)
    ).astype(ml_dtypes.bfloat16)
    eb2_h = np.ascontiguousarray(
        ebT[:, 128:NT, :].transpose(1, 0, 2).reshape(NT2, H * NT)
    ).astype(ml_dtypes.bfloat16)

    qkv_bias_nonzero = bool(q_bias.any() or v_bias.any())
    proj_bias_nonzero = bool(proj_b.any())

    in_maps = []
    for i in range(NCORES):
        xs = x[i * BPC:(i + 1) * BPC].reshape(TOK, DIM)
        m = {
            "xt": np.ascontiguousarray(xs.T).astype(ml_dtypes.bfloat16),
            "qkw": qkw_h, "vw": vw_h, "pw": pw_h,
            "eb1": eb1_h, "eb2": eb2_h,
        }
        if qkv_bias_nonzero:
            m["qkb"] = np.ascontiguousarray(
                np.concatenate([q_bias * np.float32(SCALE),
                                np.zeros_like(q_bias)])[None, :],
                dtype=np.float32).astype(ml_dtypes.bfloat16)
            m["vb"] = np.ascontiguousarray(
                v_bias[None, :]).astype(ml_dtypes.bfloat16)
        if proj_bias_nonzero:
            m["pb"] = np.ascontiguousarray(
                proj_b[None, :], dtype=np.float32).astype(ml_dtypes.bfloat16)
        in_maps.append(m)
    return in_maps, qkv_bias_nonzero, proj_bias_nonzero


def kernel(x, qkv_w, q_bias, v_bias, rpb_table, proj_w, proj_b, rel_index):
    in_maps, qb_nz, pb_nz = prepare_inputs(
        x, qkv_w, q_bias, v_bias, rpb_table, proj_w, proj_b, rel_index)
    nc = _built(qb_nz, pb_nz)
    res = run_bass_kernel_spmd(nc, in_maps, core_ids=list(range(NCORES)))
    outs = []
    for i in range(NCORES):
        ofm = res.results[i]["out"]                  # [768, 1576]
        outs.append(ofm.T.reshape(BPC, NT, DIM))
    return np.concatenate(outs, axis=0).astype(np.float32)


# revision 15
# speedup vs baseline: 1.1203x; 1.0078x over previous
"""Trainium2 Bass kernel for windowed multi-head attention (nn_AttentionWindow).

Reference computation (B=64, N=197, DIM=768, H=12, HD=64):
    qkv  = x @ qkv_w.T + [q_bias, 0, v_bias]
    q, k, v = split(qkv);  q *= HD**-0.5
    attn = softmax(q @ k.T + rpb_table[rel_index])
    out  = (attn @ v) @ proj_w.T + proj_b

Sharding: data-parallel over batch across 8 NeuronCores (8 batches/core).

Per-core design (bf16 matmuls on TensorE, fp32 PSUM accumulation):
  - x pre-transposed on host to xT [768, 1576] bf16 (feature-major),
    resident in SBUF. q,k computed feature-major into one resident
    [128, 12, 1576] tile, tiled over 512-token slices (batch-agnostic).
  - v: token-major per batch ([tokens, channels], 128+69 row chunks) so
    the attention contraction has tokens on partitions; stored bf16.
  - Scores transposed: S^T[j,i] = k_h[:,j]^T q_h, per (head-pair,
    batch) item. The two heads of a pair use opposite 64-row halves of
    the PE array (row-packing -> concurrent matmuls). Softmax WITHOUT
    max subtraction (scores are O(1): q pre-scaled by 1/8): exp on
    ScalarE (both heads' score chunks merged into one strided
    activation) -> P^T bf16, then bias multiply against the precomputed
    exp(bias) table (j1 chunk on VectorE, j2 chunk on GpSimd):
    softmax(S+B) = expS*expB / sums.
  - O^T[d,i] = sum_j v[j,d] P^T[j,i]: head pair col-packed via
    tile_position (0,0)/(0,64); softmax denominators via ones[128,64]
    matmuls, ALSO col-packed by head into the same PSUM tile as O^T
    (cols 256:453) -> one reciprocal_approx_fast + one merged
    normalization multiply per item on VectorE.
  - Schedule is batch-major: qk chunks first, then per batch
    v[b] -> 6 attention items -> proj[b]; the v/proj matmul streams
    fill TensorE during softmax latencies and output DMA overlaps.
  - PSUM->SBUF copies (qk, v, proj) run on GpSimd, freeing ScalarE for
    the exps and VectorE for recip/normalize.
"""
import sys
import functools

sys.path.insert(0, "/opt/trn_rl_repo")

import numpy as np
import ml_dtypes

import concourse.bass as bass  # noqa: E402
import concourse.bacc as bacc  # noqa: E402
import concourse.mybir as mybir  # noqa: E402
from concourse.tile import TileContext  # noqa: E402
from concourse.bass_utils import run_bass_kernel_spmd  # noqa: E402

F32 = mybir.dt.float32
BF16 = mybir.dt.bfloat16

NCORES = 8
B, NT, DIM = 64, 197, 768
H, HD = 12, 64
SCALE = HD ** -0.5  # 0.125, exact power of two -> folded into q weights
BPC = B // NCORES   # 8 batches per core
TOK = BPC * NT      # 1576 tokens per core
KC = DIM // 128     # 6
NT2 = NT - 128      # 69 (second token chunk)
SKEW = 2            # attention software-pipeline depth (items)
# 512-token slices for the token-parallel qk matmul
SLICES = [(s * 512, min(TOK, (s + 1) * 512)) for s in range((TOK + 511) // 512)]


def build(qkv_bias_nonzero: bool, proj_bias_nonzero: bool):
    nc = bacc.Bacc("TRN2", target_bir_lowering=False, debug=False)

    xt = nc.dram_tensor("xt", [DIM, TOK], BF16, kind="ExternalInput")
    qkw = nc.dram_tensor("qkw", [DIM, 2 * DIM], BF16, kind="ExternalInput")
    vw = nc.dram_tensor("vw", [DIM, DIM], BF16, kind="ExternalInput")
    pw = nc.dram_tensor("pw", [DIM, DIM], BF16, kind="ExternalInput")
    eb1 = nc.dram_tensor("eb1", [128, H * NT], BF16, kind="ExternalInput")
    eb2 = nc.dram_tensor("eb2", [NT2, H * NT], BF16, kind="ExternalInput")
    out = nc.dram_tensor("out", [DIM, TOK], F32, kind="ExternalOutput")
    if qkv_bias_nonzero:
        qkb = nc.dram_tensor("qkb", [1, 2 * DIM], BF16, kind="ExternalInput")
        vb = nc.dram_tensor("vb", [1, DIM], BF16, kind="ExternalInput")
    if proj_bias_nonzero:
        pb = nc.dram_tensor("pb", [1, DIM], BF16, kind="ExternalInput")

    with TileContext(nc) as tc:
        with (
            tc.tile_pool(name="const", bufs=1) as constp,
            tc.tile_pool(name="vp", bufs=6) as vp,
            tc.tile_pool(name="pp", bufs=2 * (SKEW + 2)) as pp,
            tc.tile_pool(name="rcp", bufs=3) as rcp,
            tc.tile_pool(name="obp", bufs=4) as obp,
            tc.tile_pool(name="mm", bufs=2, space="PSUM") as mm,
            tc.tile_pool(name="ots", bufs=2, space="PSUM") as ots,
            tc.tile_pool(name="sta", bufs=2, space="PSUM") as sta,
        ):
            # ---- resident constants & activations ----
            # DMA order matters: qk needs xb+qkw first; vw next (v of
            # batch 0 runs right after qk); eb tables before the first
            # bias-multiply; pw last (proj is latest consumer).
            xb_s = constp.tile([128, KC, TOK], BF16, name="xb_s")
            qkw_s = constp.tile([128, KC, 2 * DIM], BF16, name="qkw_s")
            vw_s = constp.tile([128, KC, DIM], BF16, name="vw_s")
            pw_s = constp.tile([128, KC, DIM], BF16, name="pw_s")
            # all input DMAs on the sync hardware-DGE queue (engine-issued
            # DMAs fall back to slow software-dynamic queues); xb/qkw
            # chunks interleaved so qk compute starts feeding ASAP
            for kc in range(KC):
                nc.sync.dma_start(xb_s[:, kc, :], xt[kc * 128:(kc + 1) * 128, :])
                nc.sync.dma_start(qkw_s[:, kc, 0:DIM],
                                  qkw[kc * 128:(kc + 1) * 128, 0:DIM])
            for kc in range(KC):
                nc.sync.dma_start(qkw_s[:, kc, DIM:2 * DIM],
                                  qkw[kc * 128:(kc + 1) * 128, DIM:2 * DIM])
            for kc in range(KC):
                nc.sync.dma_start(vw_s[:, kc, :], vw[kc * 128:(kc + 1) * 128, :])
            eb1_s = constp.tile([128, H * NT], BF16, name="eb1_s")
            eb2_s = constp.tile([NT2, H * NT], BF16, name="eb2_s")
            nc.sync.dma_start(eb1_s[:, :], eb1[:, :])
            nc.sync.dma_start(eb2_s[:, :], eb2[:, :])
            for kc in range(KC):
                nc.sync.dma_start(pw_s[:, kc, :], pw[kc * 128:(kc + 1) * 128, :])
            # ones [128, 64] lhsT for the col-packed denominator matmuls
            ones64 = constp.tile([128, 64], BF16, name="ones64")
            nc.gpsimd.memset(ones64[:, :], 1.0)
            # big resident activations: q,k and proj-rhs (bf16)
            qk_s = constp.tile([128, 2 * KC, TOK], BF16, name="qk_s")
            op_s = constp.tile([128, KC, TOK], BF16, name="op_s")
            if qkv_bias_nonzero:
                qkb_s = constp.tile([1, 2 * DIM], BF16, name="qkb_s")
                vb_s = constp.tile([1, DIM], BF16, name="vb_s")
                nc.sync.dma_start(qkb_s[:, :], qkb[:, :])
                nc.sync.dma_start(vb_s[:, :], vb[:, :])
            if proj_bias_nonzero:
                pb_s = constp.tile([1, DIM], BF16, name="pb_s")
                nc.sync.dma_start(pb_s[:, :], pb[:, :])
            if qkv_bias_nonzero or proj_bias_nonzero:
                ones_bfr = constp.tile([1, 512], BF16, name="ones_bfr")
                nc.gpsimd.memset(ones_bfr[:, :], 1.0)

            # ---- q,k feature-major: 12 channel-chunks x token slices ----
            for c in range(2 * KC):
                for t0, t1 in SLICES:
                    acc = mm.tile([128, 512], F32, name="acc_qk", tag="mm")
                    w = t1 - t0
                    for kc in range(KC):
                        nc.tensor.matmul(
                            acc[:, 0:w],
                            qkw_s[:, kc, c * 128:(c + 1) * 128],
                            xb_s[:, kc, t0:t1],
                            start=(kc == 0),
                            stop=(kc == KC - 1) and not qkv_bias_nonzero,
                        )
                    if qkv_bias_nonzero:
                        nc.tensor.matmul(
                            acc[:, 0:w],
                            qkb_s[0:1, c * 128:(c + 1) * 128],
                            ones_bfr[0:1, 0:w],
                            start=False, stop=True,
                        )
                    # PSUM->SBUF: alternate scalar/vector (both idle here)
                    if (c * len(SLICES) + (t0 // 512)) % 2 == 0:
                        nc.scalar.copy(qk_s[:, c, t0:t1], acc[:, 0:w])
                    else:
                        nc.vector.tensor_copy(qk_s[:, c, t0:t1], acc[:, 0:w])

            # ---- v token-major for one batch: [(128|69) tok, 768 ch] ----
            vt = [[None, None] for _ in range(BPC)]

            def emit_v(b):
                for tch in range(2):
                    toff = b * NT + tch * 128
                    tlen = 128 if tch == 0 else NT2
                    t = vp.tile([128, DIM], BF16, name="v_t", tag="v")
                    for half in range(2):
                        n0, n1 = half * 384, (half + 1) * 384
                        acc = mm.tile([128, 384], F32, name="acc_v", tag="mm")
                        for kc in range(KC):
                            nc.tensor.matmul(
                                acc[0:tlen, :],
                                xb_s[:, kc, toff:toff + tlen],
                                vw_s[:, kc, n0:n1],
                                start=(kc == 0),
                                stop=(kc == KC - 1) and not qkv_bias_nonzero,
                            )
                        if qkv_bias_nonzero:
                            nc.tensor.matmul(
                                acc[0:tlen, :],
                                ones_bfr[0:1, 0:tlen],
                                vb_s[0:1, n0:n1],
                                start=False, stop=True,
                            )
                        if half == 0:
                            nc.scalar.copy(t[0:tlen, n0:n1], acc[0:tlen, :])
                        else:
                            nc.vector.tensor_copy(t[0:tlen, n0:n1],
                                                  acc[0:tlen, :])
                    vt[b][tch] = t

            # ---- attention stages (per (batch, head-pair) item) ----
            def stage_a(b, hp):
                """Scores for heads 2hp,2hp+1 (array row-packed), merged
                exp + bias-mult (j1 on vector, j2 on gpsimd) -> P^T."""
                h0 = 2 * hp
                st = sta.tile([128, 1024], F32, name="st", tag="sta")
                q0 = qk_s[0:64, hp, b * NT:(b + 1) * NT]
                q1 = qk_s[64:128, hp, b * NT:(b + 1) * NT]
                # j1 chunks (K rows 0:64 then 64:128 -> packed), then j2
                nc.tensor.matmul(st[:, 0:NT],
                                 qk_s[0:64, KC + hp, b * NT:b * NT + 128],
                                 q0, start=True, stop=True)
                nc.tensor.matmul(st[:, 512:512 + NT],
                                 qk_s[64:128, KC + hp, b * NT:b * NT + 128],
                                 q1, start=True, stop=True)
                nc.tensor.matmul(st[0:NT2, 256:256 + NT],
                                 qk_s[0:64, KC + hp, b * NT + 128:(b + 1) * NT],
                                 q0, start=True, stop=True)
                nc.tensor.matmul(st[0:NT2, 768:768 + NT],
                                 qk_s[64:128, KC + hp, b * NT + 128:(b + 1) * NT],
                                 q1, start=True, stop=True)
                # merged exp over both heads (strided free AP), bf16 out
                pj1 = pp.tile([128, 2, NT], BF16, name="pj1", tag="p")
                nc.scalar.activation(
                    pj1[:, :, :], st[:, 0:1024].rearrange("p (h x) -> p h x", h=2)[:, :, 0:NT],
                    mybir.ActivationFunctionType.Exp)
                pj2 = pp.tile([128, 2, NT], BF16, name="pj2", tag="p")
                nc.scalar.activation(
                    pj2[0:NT2, :, :],
                    st[0:NT2, 0:1024].rearrange("p (h x) -> p h x", h=2)[:, :, 256:256 + NT],
                    mybir.ActivationFunctionType.Exp)
                # bias multiply (heads adjacent in eb tables)
                nc.vector.tensor_mul(
                    pj1[:, :, :].rearrange("p h x -> p (h x)"),
                    pj1[:, :, :].rearrange("p h x -> p (h x)"),
                    eb1_s[:, h0 * NT:(h0 + 2) * NT])
                nc.vector.tensor_mul(
                    pj2[0:NT2, :, :].rearrange("p h x -> p (h x)"),
                    pj2[0:NT2, :, :].rearrange("p h x -> p (h x)"),
                    eb2_s[:, h0 * NT:(h0 + 2) * NT])
                return pj1, pj2

            def stage_b(b, hp, pj1, pj2):
                """O^T and denominators, both head-pair col-packed into
                one PSUM tile; one recip + one merged normalize."""
                h0, h1 = 2 * hp, 2 * hp + 1
                ot = ots.tile([128, 512], F32, name="ot", tag="ot")
                nc.tensor.matmul(
                    ot[0:64, 0:NT],
                    vt[b][0][:, h0 * HD:(h0 + 1) * HD],
                    pj1[:, 0, :], start=True, stop=False)
                nc.tensor.matmul(
                    ot[64:128, 0:NT],
                    vt[b][0][:, h1 * HD:(h1 + 1) * HD],
                    pj1[:, 1, :], start=True, stop=False,
                    tile_position=(0, 64))
                nc.tensor.matmul(
                    ot[0:64, 0:NT],
                    vt[b][1][0:NT2, h0 * HD:(h0 + 1) * HD],
                    pj2[0:NT2, 0, :], start=False, stop=True)
                nc.tensor.matmul(
                    ot[64:128, 0:NT],
                    vt[b][1][0:NT2, h1 * HD:(h1 + 1) * HD],
                    pj2[0:NT2, 1, :], start=False, stop=True,
                    tile_position=(0, 64))
                # denominators: ones[128,64] lhsT, heads col-packed into
                # cols 256:256+NT of the same tile (partitions 0:64 = h0,
                # 64:128 = h1); accumulation groups in one PSUM bank must
                # stay sequential (interleaving open groups corrupts them)
                nc.tensor.matmul(
                    ot[0:64, 256:256 + NT], ones64[:, :],
                    pj1[:, 0, :], start=True, stop=False)
                nc.tensor.matmul(
                    ot[64:128, 256:256 + NT], ones64[:, :],
                    pj1[:, 1, :], start=True, stop=False,
                    tile_position=(0, 64))
                nc.tensor.matmul(
                    ot[0:64, 256:256 + NT], ones64[0:NT2, :],
                    pj2[0:NT2, 0, :], start=False, stop=True)
                nc.tensor.matmul(
                    ot[64:128, 256:256 + NT], ones64[0:NT2, :],
                    pj2[0:NT2, 1, :], start=False, stop=True,
                    tile_position=(0, 64))
                rc = rcp.tile([128, NT], F32, name="rc", tag="rc")
                nc.vector.reciprocal_approx_fast(
                    out=rc[:, :], in_=ot[:, 256:256 + NT])
                nc.vector.tensor_mul(
                    op_s[:, hp, b * NT:(b + 1) * NT],
                    ot[:, 0:NT], rc[:, :])

            # ---- projection over a token range (1-2 batches) ----
            def emit_proj(t0, t1):
                w = t1 - t0
                for c in range(KC):
                    acc = mm.tile([128, 512], F32, name="acc_p", tag="mm")
                    for kp in range(KC):
                        nc.tensor.matmul(
                            acc[:, 0:w],
                            pw_s[:, kp, c * 128:(c + 1) * 128],
                            op_s[:, kp, t0:t1],
                            start=(kp == 0),
                            stop=(kp == KC - 1) and not proj_bias_nonzero,
                        )
                    if proj_bias_nonzero:
                        nc.tensor.matmul(
                            acc[:, 0:w],
                            pb_s[0:1, c * 128:(c + 1) * 128],
                            ones_bfr[0:1, 0:w],
                            start=False, stop=True,
                        )
                    obt = obp.tile([128, 2 * NT], F32, name="obt", tag="ob")
                    if c % 2 == 0:
                        nc.vector.tensor_copy(obt[:, 0:w], acc[:, 0:w])
                    else:
                        nc.scalar.copy(obt[:, 0:w], acc[:, 0:w])
                    nc.sync.dma_start(out[c * 128:(c + 1) * 128, t0:t1],
                                      obt[:, 0:w])

            # ---- batch-major schedule: v[b] -> items -> proj[b] ----
            items = [(b, hp) for b in range(BPC) for hp in range(KC)]
            pend = {}

            def run_stage_b(i):
                b, hp = items[i]
                stage_b(b, hp, *pend.pop(i))
                if hp == KC - 1:
                    # batches 0-5 project in pairs (394-wide rhs halves
                    # per-matmul overhead); 6 and 7 go solo so the last
                    # batch's projection isn't delayed into the tail
                    if b in (1, 3, 5):
                        emit_proj((b - 1) * NT, (b + 1) * NT)
                    elif b >= 6:
                        emit_proj(b * NT, (b + 1) * NT)

            for i, (b, hp) in enumerate(items):
                if hp == 0:
                    emit_v(b)
                pend[i] = stage_a(b, hp)
                if i >= SKEW:
                    run_stage_b(i - SKEW)
            for i in range(len(items) - SKEW, len(items)):
                run_stage_b(i)

    nc.compile()
    return nc


@functools.lru_cache(maxsize=4)
def _built(qkv_bias_nonzero: bool, proj_bias_nonzero: bool):
    return build(qkv_bias_nonzero, proj_bias_nonzero)


def prepare_inputs(x, qkv_w, q_bias, v_bias, rpb_table, proj_w, proj_b, rel_index):
    """Host-side prep: shard + transpose + fold scale + gather bias table."""
    x = np.asarray(x, dtype=np.float32)
    qkv_w = np.asarray(qkv_w, dtype=np.float32)
    q_bias = np.asarray(q_bias, dtype=np.float32)
    v_bias = np.asarray(v_bias, dtype=np.float32)
    rpb_table = np.asarray(rpb_table, dtype=np.float32)
    proj_w = np.asarray(proj_w, dtype=np.float32)
    proj_b = np.asarray(proj_b, dtype=np.float32)
    rel_index = np.asarray(rel_index)

    qw = qkv_w[0:DIM] * np.float32(SCALE)   # exact: SCALE is a power of two
    qkw_h = np.ascontiguousarray(
        np.concatenate([qw, qkv_w[DIM:2 * DIM]], axis=0).T).astype(
        ml_dtypes.bfloat16)                                      # [768, 1536]
    vw_h = np.ascontiguousarray(qkv_w[2 * DIM:3 * DIM].T).astype(
        ml_dtypes.bfloat16)                                      # [768, 768]
    pw_h = np.ascontiguousarray(proj_w.T).astype(ml_dtypes.bfloat16)

    # bias[i, j, h] -> exp -> ebT[h, j, i]
    bias = rpb_table[rel_index]                                  # (197,197,12)
    ebT = np.exp(bias.astype(np.float32)).transpose(2, 1, 0)     # (12, j, i)
    eb1_h = np.ascontiguousarray(
        ebT[:, 0:128, :].transpose(1, 0, 2).reshape(128, H * NT)
    ).astype(ml_dtypes.bfloat16)
    eb2_h = np.ascontiguousarray(
        ebT[:, 128:NT, :].transpose(1, 0, 2).reshape(NT2, H * NT)
    ).astype(ml_dtypes.bfloat16)

    qkv_bias_nonzero = bool(q_bias.any() or v_bias.any())
    proj_bias_nonzero = bool(proj_b.any())

    in_maps = []
    for i in range(NCORES):
        xs = x[i * BPC:(i + 1) * BPC].reshape(TOK, DIM)
        m = {
            "xt": np.ascontiguousarray(xs.T).astype(ml_dtypes.bfloat16),
            "qkw": qkw_h, "vw": vw_h, "pw": pw_h,
            "eb1": eb1_h, "eb2": eb2_h,
        }
        if qkv_bias_nonzero:
            m["qkb"] = np.ascontiguousarray(
                np.concatenate([q_bias * np.float32(SCALE),
                                np.zeros_like(q_bias)])[None, :],
                dtype=np.float32).astype(ml_dtypes.bfloat16)
            m["vb"] = np.ascontiguousarray(
                v_bias[None, :]).astype(ml_dtypes.bfloat16)
        if proj_bias_nonzero:
            m["pb"] = np.ascontiguousarray(
                proj_b[None, :], dtype=np.float32).astype(ml_dtypes.bfloat16)
        in_maps.append(m)
    return in_maps, qkv_bias_nonzero, proj_bias_nonzero


def kernel(x, qkv_w, q_bias, v_bias, rpb_table, proj_w, proj_b, rel_index):
    in_maps, qb_nz, pb_nz = prepare_inputs(
        x, qkv_w, q_bias, v_bias, rpb_table, proj_w, proj_b, rel_index)
    nc = _built(qb_nz, pb_nz)
    res = run_bass_kernel_spmd(nc, in_maps, core_ids=list(range(NCORES)))
    outs = []
    for i in range(NCORES):
        ofm = res.results[i]["out"]                  # [768, 1576]
        outs.append(ofm.T.reshape(BPC, NT, DIM))
    return np.concatenate(outs, axis=0).astype(np.float32)


# revision 20
# speedup vs baseline: 1.1300x; 1.0087x over previous
"""Trainium2 Bass kernel for windowed multi-head attention (nn_AttentionWindow).

Reference computation (B=64, N=197, DIM=768, H=12, HD=64):
    qkv  = x @ qkv_w.T + [q_bias, 0, v_bias]
    q, k, v = split(qkv);  q *= HD**-0.5
    attn = softmax(q @ k.T + rpb_table[rel_index])
    out  = (attn @ v) @ proj_w.T + proj_b

Sharding: data-parallel over batch across 8 NeuronCores (8 batches/core).

Per-core design (bf16 matmuls on TensorE, fp32 PSUM accumulation):
  - x pre-transposed on host to xT [768, 1576] bf16 (feature-major),
    resident in SBUF. q,k computed feature-major into one resident
    [128, 12, 1576] tile, tiled over 512-token slices (batch-agnostic).
  - v: token-major per batch ([tokens, channels], 128+69 row chunks) so
    the attention contraction has tokens on partitions; stored bf16.
  - Scores transposed: S^T[j,i] = k_h[:,j]^T q_h, per (head-pair,
    batch) item. The two heads of a pair use opposite 64-row halves of
    the PE array (row-packing -> concurrent matmuls). Softmax WITHOUT
    max subtraction (scores are O(1): q pre-scaled by 1/8): exp on
    ScalarE (both heads' score chunks merged into one strided
    activation) -> P^T bf16, then bias multiply against the precomputed
    exp(bias) table (j1 chunk on VectorE, j2 chunk on GpSimd):
    softmax(S+B) = expS*expB / sums.
  - O^T[d,i] = sum_j v[j,d] P^T[j,i]: head pair col-packed via
    tile_position (0,0)/(0,64); softmax denominators via ones[128,64]
    matmuls, ALSO col-packed by head into the same PSUM tile as O^T
    (cols 256:453) -> one reciprocal_approx_fast + one merged
    normalization multiply per item on VectorE.
  - Schedule is batch-major: qk chunks first, then per batch
    v[b] -> 6 attention items -> proj[b]; the v/proj matmul streams
    fill TensorE during softmax latencies and output DMA overlaps.
  - PSUM->SBUF copies (qk, v, proj) run on GpSimd, freeing ScalarE for
    the exps and VectorE for recip/normalize.
"""
import sys
import functools

sys.path.insert(0, "/opt/trn_rl_repo")

import numpy as np
import ml_dtypes

import concourse.bass as bass  # noqa: E402
import concourse.bacc as bacc  # noqa: E402
import concourse.mybir as mybir  # noqa: E402
from concourse.tile import TileContext  # noqa: E402
from concourse.bass_utils import run_bass_kernel_spmd  # noqa: E402

F32 = mybir.dt.float32
BF16 = mybir.dt.bfloat16

NCORES = 8
B, NT, DIM = 64, 197, 768
H, HD = 12, 64
SCALE = HD ** -0.5  # 0.125, exact power of two -> folded into q weights
BPC = B // NCORES   # 8 batches per core
TOK = BPC * NT      # 1576 tokens per core
KC = DIM // 128     # 6
NT2 = NT - 128      # 69 (second token chunk)
SKEW = 2            # attention software-pipeline depth (items)
# token slices for the token-parallel qk matmul (PSUM bank limit 512);
# the tail is split 276+276 rather than 512+40 so no matmul is so short
# that its weight load dominates
SLICES = [(0, 512), (512, 1024), (1024, 1300), (1300, TOK)]


def build(qkv_bias_nonzero: bool, proj_bias_nonzero: bool):
    nc = bacc.Bacc("TRN2", target_bir_lowering=False, debug=False)

    xt = nc.dram_tensor("xt", [DIM, TOK], BF16, kind="ExternalInput")
    qkw = nc.dram_tensor("qkw", [DIM, 2 * DIM], BF16, kind="ExternalInput")
    vw = nc.dram_tensor("vw", [DIM, DIM], BF16, kind="ExternalInput")
    pw = nc.dram_tensor("pw", [DIM, DIM], BF16, kind="ExternalInput")
    eb1 = nc.dram_tensor("eb1", [128, H * NT], BF16, kind="ExternalInput")
    eb2 = nc.dram_tensor("eb2", [NT2, H * NT], BF16, kind="ExternalInput")
    out = nc.dram_tensor("out", [DIM, TOK], BF16, kind="ExternalOutput")
    if qkv_bias_nonzero:
        qkb = nc.dram_tensor("qkb", [1, 2 * DIM], BF16, kind="ExternalInput")
        vb = nc.dram_tensor("vb", [1, DIM], BF16, kind="ExternalInput")
    if proj_bias_nonzero:
        pb = nc.dram_tensor("pb", [1, DIM], BF16, kind="ExternalInput")

    with TileContext(nc) as tc:
        with (
            tc.tile_pool(name="const", bufs=1) as constp,
            tc.tile_pool(name="vp", bufs=6) as vp,
            tc.tile_pool(name="pp", bufs=2 * (SKEW + 2)) as pp,
            tc.tile_pool(name="rcp", bufs=3) as rcp,
            tc.tile_pool(name="obp", bufs=4) as obp,
            tc.tile_pool(name="mm", bufs=2, space="PSUM") as mm,
            tc.tile_pool(name="ots", bufs=2, space="PSUM") as ots,
            tc.tile_pool(name="sta", bufs=2, space="PSUM") as sta,
        ):
            # ---- resident constants & activations ----
            # DMA order matters: qk needs xb+qkw first; vw next (v of
            # batch 0 runs right after qk); eb tables before the first
            # bias-multiply; pw last (proj is latest consumer).
            xb_s = constp.tile([128, KC, TOK], BF16, name="xb_s")
            qkw_s = constp.tile([128, KC, 2 * DIM], BF16, name="qkw_s")
            vw_s = constp.tile([128, KC, DIM], BF16, name="vw_s")
            pw_s = constp.tile([128, KC, DIM], BF16, name="pw_s")
            # all input DMAs on the sync hardware-DGE queue (engine-issued
            # DMAs fall back to slow software-dynamic queues); xb/qkw
            # chunks interleaved so qk compute starts feeding ASAP
            for kc in range(KC):
                nc.sync.dma_start(xb_s[:, kc, :], xt[kc * 128:(kc + 1) * 128, :])
                nc.sync.dma_start(qkw_s[:, kc, 0:DIM],
                                  qkw[kc * 128:(kc + 1) * 128, 0:DIM])
            for kc in range(KC):
                nc.sync.dma_start(qkw_s[:, kc, DIM:2 * DIM],
                                  qkw[kc * 128:(kc + 1) * 128, DIM:2 * DIM])
            for kc in range(KC):
                nc.sync.dma_start(vw_s[:, kc, :], vw[kc * 128:(kc + 1) * 128, :])
            eb1_s = constp.tile([128, H * NT], BF16, name="eb1_s")
            eb2_s = constp.tile([NT2, H * NT], BF16, name="eb2_s")
            nc.sync.dma_start(eb1_s[:, :], eb1[:, :])
            nc.sync.dma_start(eb2_s[:, :], eb2[:, :])
            for kc in range(KC):
                nc.sync.dma_start(pw_s[:, kc, :], pw[kc * 128:(kc + 1) * 128, :])
            # ones [128, 64] lhsT for the col-packed denominator matmuls
            ones64 = constp.tile([128, 64], BF16, name="ones64")
            nc.gpsimd.memset(ones64[:, :], 1.0)
            # big resident activations: q,k and proj-rhs (bf16)
            qk_s = constp.tile([128, 2 * KC, TOK], BF16, name="qk_s")
            op_s = constp.tile([128, KC, TOK], BF16, name="op_s")
            if qkv_bias_nonzero:
                qkb_s = constp.tile([1, 2 * DIM], BF16, name="qkb_s")
                vb_s = constp.tile([1, DIM], BF16, name="vb_s")
                nc.sync.dma_start(qkb_s[:, :], qkb[:, :])
                nc.sync.dma_start(vb_s[:, :], vb[:, :])
            if proj_bias_nonzero:
                pb_s = constp.tile([1, DIM], BF16, name="pb_s")
                nc.sync.dma_start(pb_s[:, :], pb[:, :])
            if qkv_bias_nonzero or proj_bias_nonzero:
                ones_bfr = constp.tile([1, 512], BF16, name="ones_bfr")
                nc.gpsimd.memset(ones_bfr[:, :], 1.0)

            # ---- q,k feature-major: 12 channel-chunks x token slices ----
            for c in range(2 * KC):
                for si, (t0, t1) in enumerate(SLICES):
                    acc = mm.tile([128, 512], F32, name="acc_qk", tag="mm")
                    w = t1 - t0
                    for kc in range(KC):
                        nc.tensor.matmul(
                            acc[:, 0:w],
                            qkw_s[:, kc, c * 128:(c + 1) * 128],
                            xb_s[:, kc, t0:t1],
                            start=(kc == 0),
                            stop=(kc == KC - 1) and not qkv_bias_nonzero,
                        )
                    if qkv_bias_nonzero:
                        nc.tensor.matmul(
                            acc[:, 0:w],
                            qkb_s[0:1, c * 128:(c + 1) * 128],
                            ones_bfr[0:1, 0:w],
                            start=False, stop=True,
                        )
                    # PSUM->SBUF: alternate scalar/vector (both idle here)
                    if (c * len(SLICES) + si) % 2 == 0:
                        nc.scalar.copy(qk_s[:, c, t0:t1], acc[:, 0:w])
                    else:
                        nc.vector.tensor_copy(qk_s[:, c, t0:t1], acc[:, 0:w])

            # ---- v token-major for one batch: [(128|69) tok, 768 ch] ----
            vt = [[None, None] for _ in range(BPC)]

            def emit_v(b):
                for tch in range(2):
                    toff = b * NT + tch * 128
                    tlen = 128 if tch == 0 else NT2
                    t = vp.tile([128, DIM], BF16, name="v_t", tag="v")
                    for half in range(2):
                        n0, n1 = half * 384, (half + 1) * 384
                        acc = mm.tile([128, 384], F32, name="acc_v", tag="mm")
                        for kc in range(KC):
                            nc.tensor.matmul(
                                acc[0:tlen, :],
                                xb_s[:, kc, toff:toff + tlen],
                                vw_s[:, kc, n0:n1],
                                start=(kc == 0),
                                stop=(kc == KC - 1) and not qkv_bias_nonzero,
                            )
                        if qkv_bias_nonzero:
                            nc.tensor.matmul(
                                acc[0:tlen, :],
                                ones_bfr[0:1, 0:tlen],
                                vb_s[0:1, n0:n1],
                                start=False, stop=True,
                            )
                        if half == 0:
                            nc.scalar.copy(t[0:tlen, n0:n1], acc[0:tlen, :])
                        else:
                            nc.vector.tensor_copy(t[0:tlen, n0:n1],
                                                  acc[0:tlen, :])
                    vt[b][tch] = t

            # ---- attention stages (per (batch, head-pair) item) ----
            def stage_a(b, hp):
                """Scores for heads 2hp,2hp+1 (array row-packed), merged
                exp + bias-mult (j1 on vector, j2 on gpsimd) -> P^T."""
                h0 = 2 * hp
                st = sta.tile([128, 1024], F32, name="st", tag="sta")
                q0 = qk_s[0:64, hp, b * NT:(b + 1) * NT]
                q1 = qk_s[64:128, hp, b * NT:(b + 1) * NT]
                # j1 chunks (K rows 0:64 then 64:128 -> packed), then j2
                nc.tensor.matmul(st[:, 0:NT],
                                 qk_s[0:64, KC + hp, b * NT:b * NT + 128],
                                 q0, start=True, stop=True)
                nc.tensor.matmul(st[:, 512:512 + NT],
                                 qk_s[64:128, KC + hp, b * NT:b * NT + 128],
                                 q1, start=True, stop=True)
                nc.tensor.matmul(st[0:NT2, 256:256 + NT],
                                 qk_s[0:64, KC + hp, b * NT + 128:(b + 1) * NT],
                                 q0, start=True, stop=True)
                nc.tensor.matmul(st[0:NT2, 768:768 + NT],
                                 qk_s[64:128, KC + hp, b * NT + 128:(b + 1) * NT],
                                 q1, start=True, stop=True)
                # merged exp over both heads (strided free AP), bf16 out
                pj1 = pp.tile([128, 2, NT], BF16, name="pj1", tag="p")
                nc.scalar.activation(
                    pj1[:, :, :], st[:, 0:1024].rearrange("p (h x) -> p h x", h=2)[:, :, 0:NT],
                    mybir.ActivationFunctionType.Exp)
                pj2 = pp.tile([128, 2, NT], BF16, name="pj2", tag="p")
                nc.scalar.activation(
                    pj2[0:NT2, :, :],
                    st[0:NT2, 0:1024].rearrange("p (h x) -> p h x", h=2)[:, :, 256:256 + NT],
                    mybir.ActivationFunctionType.Exp)
                # bias multiply (heads adjacent in eb tables)
                nc.vector.tensor_mul(
                    pj1[:, :, :].rearrange("p h x -> p (h x)"),
                    pj1[:, :, :].rearrange("p h x -> p (h x)"),
                    eb1_s[:, h0 * NT:(h0 + 2) * NT])
                nc.vector.tensor_mul(
                    pj2[0:NT2, :, :].rearrange("p h x -> p (h x)"),
                    pj2[0:NT2, :, :].rearrange("p h x -> p (h x)"),
                    eb2_s[:, h0 * NT:(h0 + 2) * NT])
                return pj1, pj2

            def stage_b(b, hp, pj1, pj2):
                """O^T and denominators, both head-pair col-packed into
                one PSUM tile; one recip + one merged normalize."""
                h0, h1 = 2 * hp, 2 * hp + 1
                ot = ots.tile([128, 512], F32, name="ot", tag="ot")
                nc.tensor.matmul(
                    ot[0:64, 0:NT],
                    vt[b][0][:, h0 * HD:(h0 + 1) * HD],
                    pj1[:, 0, :], start=True, stop=False)
                nc.tensor.matmul(
                    ot[64:128, 0:NT],
                    vt[b][0][:, h1 * HD:(h1 + 1) * HD],
                    pj1[:, 1, :], start=True, stop=False,
                    tile_position=(0, 64))
                nc.tensor.matmul(
                    ot[0:64, 0:NT],
                    vt[b][1][0:NT2, h0 * HD:(h0 + 1) * HD],
                    pj2[0:NT2, 0, :], start=False, stop=True)
                nc.tensor.matmul(
                    ot[64:128, 0:NT],
                    vt[b][1][0:NT2, h1 * HD:(h1 + 1) * HD],
                    pj2[0:NT2, 1, :], start=False, stop=True,
                    tile_position=(0, 64))
                # denominators: ones[128,64] lhsT, heads col-packed into
                # cols 256:256+NT of the same tile (partitions 0:64 = h0,
                # 64:128 = h1); accumulation groups in one PSUM bank must
                # stay sequential (interleaving open groups corrupts them)
                nc.tensor.matmul(
                    ot[0:64, 256:256 + NT], ones64[:, :],
                    pj1[:, 0, :], start=True, stop=False)
                nc.tensor.matmul(
                    ot[64:128, 256:256 + NT], ones64[:, :],
                    pj1[:, 1, :], start=True, stop=False,
                    tile_position=(0, 64))
                nc.tensor.matmul(
                    ot[0:64, 256:256 + NT], ones64[0:NT2, :],
                    pj2[0:NT2, 0, :], start=False, stop=True)
                nc.tensor.matmul(
                    ot[64:128, 256:256 + NT], ones64[0:NT2, :],
                    pj2[0:NT2, 1, :], start=False, stop=True,
                    tile_position=(0, 64))
                rc = rcp.tile([128, NT], F32, name="rc", tag="rc")
                nc.vector.reciprocal_approx_fast(
                    out=rc[:, :], in_=ot[:, 256:256 + NT])
                nc.vector.tensor_mul(
                    op_s[:, hp, b * NT:(b + 1) * NT],
                    ot[:, 0:NT], rc[:, :])

            # ---- projection over a token range (1-2 batches) ----
            def emit_proj(t0, t1):
                w = t1 - t0
                for c in range(KC):
                    acc = mm.tile([128, 512], F32, name="acc_p", tag="mm")
                    for kp in range(KC):
                        nc.tensor.matmul(
                            acc[:, 0:w],
                            pw_s[:, kp, c * 128:(c + 1) * 128],
                            op_s[:, kp, t0:t1],
                            start=(kp == 0),
                            stop=(kp == KC - 1) and not proj_bias_nonzero,
                        )
                    if proj_bias_nonzero:
                        nc.tensor.matmul(
                            acc[:, 0:w],
                            pb_s[0:1, c * 128:(c + 1) * 128],
                            ones_bfr[0:1, 0:w],
                            start=False, stop=True,
                        )
                    obt = obp.tile([128, 2 * NT], BF16, name="obt", tag="ob")
                    if c % 2 == 0:
                        nc.vector.tensor_copy(obt[:, 0:w], acc[:, 0:w])
                    else:
                        nc.scalar.copy(obt[:, 0:w], acc[:, 0:w])
                    nc.sync.dma_start(out[c * 128:(c + 1) * 128, t0:t1],
                                      obt[:, 0:w])

            # ---- batch-major schedule: v[b] -> items -> proj[b] ----
            items = [(b, hp) for b in range(BPC) for hp in range(KC)]
            pend = {}

            def run_stage_b(i):
                b, hp = items[i]
                stage_b(b, hp, *pend.pop(i))
                if hp == KC - 1:
                    # batches 0-5 project in pairs (394-wide rhs halves
                    # per-matmul overhead); 6 and 7 go solo so the last
                    # batch's projection isn't delayed into the tail
                    if b in (1, 3, 5):
                        emit_proj((b - 1) * NT, (b + 1) * NT)
                    elif b >= 6:
                        emit_proj(b * NT, (b + 1) * NT)

            for i, (b, hp) in enumerate(items):
                if hp == 0:
                    emit_v(b)
                pend[i] = stage_a(b, hp)
                if i >= SKEW:
                    run_stage_b(i - SKEW)
            for i in range(len(items) - SKEW, len(items)):
                run_stage_b(i)

    nc.compile()
    return nc


@functools.lru_cache(maxsize=4)
def _built(qkv_bias_nonzero: bool, proj_bias_nonzero: bool):
    return build(qkv_bias_nonzero, proj_bias_nonzero)


def prepare_inputs(x, qkv_w, q_bias, v_bias, rpb_table, proj_w, proj_b, rel_index):
    """Host-side prep: shard + transpose + fold scale + gather bias table."""
    x = np.asarray(x, dtype=np.float32)
    qkv_w = np.asarray(qkv_w, dtype=np.float32)
    q_bias = np.asarray(q_bias, dtype=np.float32)
    v_bias = np.asarray(v_bias, dtype=np.float32)
    rpb_table = np.asarray(rpb_table, dtype=np.float32)
    proj_w = np.asarray(proj_w, dtype=np.float32)
    proj_b = np.asarray(proj_b, dtype=np.float32)
    rel_index = np.asarray(rel_index)

    qw = qkv_w[0:DIM] * np.float32(SCALE)   # exact: SCALE is a power of two
    qkw_h = np.ascontiguousarray(
        np.concatenate([qw, qkv_w[DIM:2 * DIM]], axis=0).T).astype(
        ml_dtypes.bfloat16)                                      # [768, 1536]
    vw_h = np.ascontiguousarray(qkv_w[2 * DIM:3 * DIM].T).astype(
        ml_dtypes.bfloat16)                                      # [768, 768]
    pw_h = np.ascontiguousarray(proj_w.T).astype(ml_dtypes.bfloat16)

    # bias[i, j, h] -> exp -> ebT[h, j, i]
    bias = rpb_table[rel_index]                                  # (197,197,12)
    ebT = np.exp(bias.astype(np.float32)).transpose(2, 1, 0)     # (12, j, i)
    eb1_h = np.ascontiguousarray(
        ebT[:, 0:128, :].transpose(1, 0, 2).reshape(128, H * NT)
    ).astype(ml_dtypes.bfloat16)
    eb2_h = np.ascontiguousarray(
        ebT[:, 128:NT, :].transpose(1, 0, 2).reshape(NT2, H * NT)
    ).astype(ml_dtypes.bfloat16)

    qkv_bias_nonzero = bool(q_bias.any() or v_bias.any())
    proj_bias_nonzero = bool(proj_b.any())

    in_maps = []
    for i in range(NCORES):
        xs = x[i * BPC:(i + 1) * BPC].reshape(TOK, DIM)
        m = {
            "xt": np.ascontiguousarray(xs.T).astype(ml_dtypes.bfloat16),
            "qkw": qkw_h, "vw": vw_h, "pw": pw_h,
            "eb1": eb1_h, "eb2": eb2_h,
        }
        if qkv_bias_nonzero:
            m["qkb"] = np.ascontiguousarray(
                np.concatenate([q_bias * np.float32(SCALE),
                                np.zeros_like(q_bias)])[None, :],
                dtype=np.float32).astype(ml_dtypes.bfloat16)
            m["vb"] = np.ascontiguousarray(
                v_bias[None, :]).astype(ml_dtypes.bfloat16)
        if proj_bias_nonzero:
            m["pb"] = np.ascontiguousarray(
                proj_b[None, :], dtype=np.float32).astype(ml_dtypes.bfloat16)
        in_maps.append(m)
    return in_maps, qkv_bias_nonzero, proj_bias_nonzero


def kernel(x, qkv_w, q_bias, v_bias, rpb_table, proj_w, proj_b, rel_index):
    in_maps, qb_nz, pb_nz = prepare_inputs(
        x, qkv_w, q_bias, v_bias, rpb_table, proj_w, proj_b, rel_index)
    nc = _built(qb_nz, pb_nz)
    res = run_bass_kernel_spmd(nc, in_maps, core_ids=list(range(NCORES)))
    outs = []
    for i in range(NCORES):
        ofm = res.results[i]["out"]                  # [768, 1576]
        outs.append(ofm.T.reshape(BPC, NT, DIM))
    return np.concatenate(outs, axis=0).astype(np.float32)


# revision 23
# speedup vs baseline: 1.1478x; 1.0158x over previous
"""Trainium2 Bass kernel for windowed multi-head attention (nn_AttentionWindow).

Reference computation (B=64, N=197, DIM=768, H=12, HD=64):
    qkv  = x @ qkv_w.T + [q_bias, 0, v_bias]
    q, k, v = split(qkv);  q *= HD**-0.5
    attn = softmax(q @ k.T + rpb_table[rel_index])
    out  = (attn @ v) @ proj_w.T + proj_b

Sharding: data-parallel over batch across 8 NeuronCores (8 batches/core).

Per-core design (bf16 matmuls on TensorE, fp32 PSUM accumulation):
  - x pre-transposed on host to xT [768, 1576] bf16 (feature-major),
    resident in SBUF. q,k computed feature-major into one resident
    [128, 12, 1576] tile, tiled over 512-token slices (batch-agnostic).
  - v: token-major per batch ([tokens, channels], 128+69 row chunks) so
    the attention contraction has tokens on partitions; stored bf16.
  - Scores transposed: S^T[j,i] = k_h[:,j]^T q_h, per (head-pair,
    batch) item. The two heads of a pair use opposite 64-row halves of
    the PE array (row-packing -> concurrent matmuls). Softmax WITHOUT
    max subtraction (scores are O(1): q pre-scaled by 1/8): exp on
    ScalarE (both heads' score chunks merged into one strided
    activation) -> P^T bf16, then bias multiply against the precomputed
    exp(bias) table (j1 chunk on VectorE, j2 chunk on GpSimd):
    softmax(S+B) = expS*expB / sums.
  - O^T[d,i] = sum_j v[j,d] P^T[j,i]: head pair col-packed via
    tile_position (0,0)/(0,64); softmax denominators via ones[128,64]
    matmuls, ALSO col-packed by head into the same PSUM tile as O^T
    (cols 256:453) -> one reciprocal_approx_fast + one merged
    normalization multiply per item on VectorE.
  - Schedule is batch-major: qk chunks first, then per batch
    v[b] -> 6 attention items -> proj[b]; the v/proj matmul streams
    fill TensorE during softmax latencies and output DMA overlaps.
  - PSUM->SBUF copies (qk, v, proj) run on GpSimd, freeing ScalarE for
    the exps and VectorE for recip/normalize.
"""
import sys
import functools

sys.path.insert(0, "/opt/trn_rl_repo")

import numpy as np
import ml_dtypes

import concourse.bass as bass  # noqa: E402
import concourse.bacc as bacc  # noqa: E402
import concourse.mybir as mybir  # noqa: E402
from concourse.tile import TileContext  # noqa: E402
from concourse.bass_utils import run_bass_kernel_spmd  # noqa: E402

F32 = mybir.dt.float32
BF16 = mybir.dt.bfloat16

NCORES = 8
B, NT, DIM = 64, 197, 768
H, HD = 12, 64
SCALE = HD ** -0.5  # 0.125, exact power of two -> folded into q weights
BPC = B // NCORES   # 8 batches per core
TOK = BPC * NT      # 1576 tokens per core
KC = DIM // 128     # 6
NT2 = NT - 128      # 69 (second token chunk)
SKEW = 2            # attention software-pipeline depth (items)
# token slices for the token-parallel qk matmul (PSUM bank limit 512);
# the tail is split 276+276 rather than 512+40 so no matmul is so short
# that its weight load dominates
SLICES = [(0, 512), (512, 1024), (1024, 1300), (1300, TOK)]


def build(qkv_bias_nonzero: bool, proj_bias_nonzero: bool):
    nc = bacc.Bacc("TRN2", target_bir_lowering=False, debug=False)

    xt = nc.dram_tensor("xt", [DIM, TOK], BF16, kind="ExternalInput")
    qkw = nc.dram_tensor("qkw", [DIM, 2 * DIM], BF16, kind="ExternalInput")
    vw = nc.dram_tensor("vw", [DIM, DIM], BF16, kind="ExternalInput")
    pw = nc.dram_tensor("pw", [DIM, DIM], BF16, kind="ExternalInput")
    eb1 = nc.dram_tensor("eb1", [128, H * NT], BF16, kind="ExternalInput")
    eb2 = nc.dram_tensor("eb2", [NT2, H * NT], BF16, kind="ExternalInput")
    out = nc.dram_tensor("out", [DIM, TOK], BF16, kind="ExternalOutput")
    if qkv_bias_nonzero:
        qkb = nc.dram_tensor("qkb", [1, 2 * DIM], BF16, kind="ExternalInput")
        vb = nc.dram_tensor("vb", [1, DIM], BF16, kind="ExternalInput")
    if proj_bias_nonzero:
        pb = nc.dram_tensor("pb", [1, DIM], BF16, kind="ExternalInput")

    with TileContext(nc) as tc:
        with (
            tc.tile_pool(name="const", bufs=1) as constp,
            tc.tile_pool(name="vp", bufs=8) as vp,
            tc.tile_pool(name="pp", bufs=12) as pp,
            tc.tile_pool(name="rcp", bufs=3) as rcp,
            tc.tile_pool(name="obp", bufs=4) as obp,
            tc.tile_pool(name="mm", bufs=2, space="PSUM") as mm,
            tc.tile_pool(name="ots", bufs=2, space="PSUM") as ots,
            tc.tile_pool(name="sta", bufs=2, space="PSUM") as sta,
        ):
            # ---- resident constants & activations ----
            # DMA order matters: qk needs xb+qkw first; vw next (v of
            # batch 0 runs right after qk); eb tables before the first
            # bias-multiply; pw last (proj is latest consumer).
            xb_s = constp.tile([128, KC, TOK], BF16, name="xb_s")
            qkw_s = constp.tile([128, KC, 2 * DIM], BF16, name="qkw_s")
            vw_s = constp.tile([128, KC, DIM], BF16, name="vw_s")
            pw_s = constp.tile([128, KC, DIM], BF16, name="pw_s")
            # all input DMAs on the sync hardware-DGE queue (engine-issued
            # DMAs fall back to slow software-dynamic queues); xb/qkw
            # chunks interleaved so qk compute starts feeding ASAP
            for kc in range(KC):
                nc.sync.dma_start(xb_s[:, kc, 0:1024],
                                  xt[kc * 128:(kc + 1) * 128, 0:1024])
                nc.sync.dma_start(qkw_s[:, kc, 0:DIM],
                                  qkw[kc * 128:(kc + 1) * 128, 0:DIM])
            for kc in range(KC):
                nc.sync.dma_start(xb_s[:, kc, 1024:TOK],
                                  xt[kc * 128:(kc + 1) * 128, 1024:TOK])
            for kc in range(KC):
                nc.sync.dma_start(qkw_s[:, kc, DIM:2 * DIM],
                                  qkw[kc * 128:(kc + 1) * 128, DIM:2 * DIM])
            for kc in range(KC):
                nc.sync.dma_start(vw_s[:, kc, :], vw[kc * 128:(kc + 1) * 128, :])
            eb1_s = constp.tile([128, H * NT], BF16, name="eb1_s")
            eb2_s = constp.tile([NT2, H * NT], BF16, name="eb2_s")
            nc.sync.dma_start(eb1_s[:, :], eb1[:, :])
            nc.sync.dma_start(eb2_s[:, :], eb2[:, :])
            for kc in range(KC):
                nc.sync.dma_start(pw_s[:, kc, :], pw[kc * 128:(kc + 1) * 128, :])
            # ones [128, 64] lhsT for the col-packed denominator matmuls
            ones64 = constp.tile([128, 64], BF16, name="ones64")
            nc.gpsimd.memset(ones64[:, :], 1.0)
            # big resident activations: q,k and proj-rhs (bf16)
            qk_s = constp.tile([128, 2 * KC, TOK], BF16, name="qk_s")
            op_s = constp.tile([128, KC, TOK], BF16, name="op_s")
            if qkv_bias_nonzero:
                qkb_s = constp.tile([1, 2 * DIM], BF16, name="qkb_s")
                vb_s = constp.tile([1, DIM], BF16, name="vb_s")
                nc.sync.dma_start(qkb_s[:, :], qkb[:, :])
                nc.sync.dma_start(vb_s[:, :], vb[:, :])
            if proj_bias_nonzero:
                pb_s = constp.tile([1, DIM], BF16, name="pb_s")
                nc.sync.dma_start(pb_s[:, :], pb[:, :])
            if qkv_bias_nonzero or proj_bias_nonzero:
                ones_bfr = constp.tile([1, 512], BF16, name="ones_bfr")
                nc.gpsimd.memset(ones_bfr[:, :], 1.0)

            # ---- q,k feature-major: 12 channel-chunks x token slices ----
            # chunk order follows DMA arrival: q chunks over the first
            # token half, then q over the second half, then k chunks
            qk_order = ([(c, s) for c in range(KC) for s in (0, 1)]
                        + [(c, s) for c in range(KC) for s in (2, 3)]
                        + [(c, s) for c in range(KC, 2 * KC)
                           for s in range(4)])
            for c, si in qk_order:
                    t0, t1 = SLICES[si]
                    acc = mm.tile([128, 512], F32, name="acc_qk", tag="mm")
                    w = t1 - t0
                    for kc in range(KC):
                        nc.tensor.matmul(
                            acc[:, 0:w],
                            qkw_s[:, kc, c * 128:(c + 1) * 128],
                            xb_s[:, kc, t0:t1],
                            start=(kc == 0),
                            stop=(kc == KC - 1) and not qkv_bias_nonzero,
                        )
                    if qkv_bias_nonzero:
                        nc.tensor.matmul(
                            acc[:, 0:w],
                            qkb_s[0:1, c * 128:(c + 1) * 128],
                            ones_bfr[0:1, 0:w],
                            start=False, stop=True,
                        )
                    # PSUM->SBUF: alternate scalar/vector (both idle here)
                    if (c * len(SLICES) + si) % 2 == 0:
                        nc.scalar.copy(qk_s[:, c, t0:t1], acc[:, 0:w])
                    else:
                        nc.vector.tensor_copy(qk_s[:, c, t0:t1], acc[:, 0:w])

            # ---- v token-major for one batch: [(128|69) tok, 768 ch] ----
            vt = [[None, None] for _ in range(BPC)]

            def emit_v(b):
                for tch in range(2):
                    toff = b * NT + tch * 128
                    tlen = 128 if tch == 0 else NT2
                    t = vp.tile([128, DIM], BF16, name="v_t", tag="v")
                    for half in range(2):
                        n0, n1 = half * 384, (half + 1) * 384
                        acc = mm.tile([128, 384], F32, name="acc_v", tag="mm")
                        for kc in range(KC):
                            nc.tensor.matmul(
                                acc[0:tlen, :],
                                xb_s[:, kc, toff:toff + tlen],
                                vw_s[:, kc, n0:n1],
                                start=(kc == 0),
                                stop=(kc == KC - 1) and not qkv_bias_nonzero,
                            )
                        if qkv_bias_nonzero:
                            nc.tensor.matmul(
                                acc[0:tlen, :],
                                ones_bfr[0:1, 0:tlen],
                                vb_s[0:1, n0:n1],
                                start=False, stop=True,
                            )
                        if half == 0:
                            nc.scalar.copy(t[0:tlen, n0:n1], acc[0:tlen, :])
                        else:
                            nc.vector.tensor_copy(t[0:tlen, n0:n1],
                                                  acc[0:tlen, :])
                    vt[b][tch] = t

            # ---- attention stages (per (batch, head-pair) item) ----
            def stage_a(b, hp):
                """Scores for heads 2hp,2hp+1 (array row-packed), merged
                exp + bias-mult (j1 on vector, j2 on gpsimd) -> P^T."""
                h0 = 2 * hp
                st = sta.tile([128, 1024], F32, name="st", tag="sta")
                q0 = qk_s[0:64, hp, b * NT:(b + 1) * NT]
                q1 = qk_s[64:128, hp, b * NT:(b + 1) * NT]
                # j1 chunks (K rows 0:64 then 64:128 -> packed), then j2
                nc.tensor.matmul(st[:, 0:NT],
                                 qk_s[0:64, KC + hp, b * NT:b * NT + 128],
                                 q0, start=True, stop=True)
                nc.tensor.matmul(st[:, 512:512 + NT],
                                 qk_s[64:128, KC + hp, b * NT:b * NT + 128],
                                 q1, start=True, stop=True)
                nc.tensor.matmul(st[0:NT2, 256:256 + NT],
                                 qk_s[0:64, KC + hp, b * NT + 128:(b + 1) * NT],
                                 q0, start=True, stop=True)
                nc.tensor.matmul(st[0:NT2, 768:768 + NT],
                                 qk_s[64:128, KC + hp, b * NT + 128:(b + 1) * NT],
                                 q1, start=True, stop=True)
                # merged exp over both heads (strided free AP), bf16 out
                pj1 = pp.tile([128, 2, NT], BF16, name="pj1", tag="p")
                nc.scalar.activation(
                    pj1[:, :, :], st[:, 0:1024].rearrange("p (h x) -> p h x", h=2)[:, :, 0:NT],
                    mybir.ActivationFunctionType.Exp)
                pj2 = pp.tile([128, 2, NT], BF16, name="pj2", tag="p")
                nc.scalar.activation(
                    pj2[0:NT2, :, :],
                    st[0:NT2, 0:1024].rearrange("p (h x) -> p h x", h=2)[:, :, 256:256 + NT],
                    mybir.ActivationFunctionType.Exp)
                # bias multiply (heads adjacent in eb tables)
                nc.vector.tensor_mul(
                    pj1[:, :, :].rearrange("p h x -> p (h x)"),
                    pj1[:, :, :].rearrange("p h x -> p (h x)"),
                    eb1_s[:, h0 * NT:(h0 + 2) * NT])
                nc.vector.tensor_mul(
                    pj2[0:NT2, :, :].rearrange("p h x -> p (h x)"),
                    pj2[0:NT2, :, :].rearrange("p h x -> p (h x)"),
                    eb2_s[:, h0 * NT:(h0 + 2) * NT])
                return pj1, pj2

            def stage_b(b, hp, pj1, pj2):
                """O^T and denominators, both head-pair col-packed into
                one PSUM tile; one recip + one merged normalize."""
                h0, h1 = 2 * hp, 2 * hp + 1
                ot = ots.tile([128, 512], F32, name="ot", tag="ot")
                nc.tensor.matmul(
                    ot[0:64, 0:NT],
                    vt[b][0][:, h0 * HD:(h0 + 1) * HD],
                    pj1[:, 0, :], start=True, stop=False)
                nc.tensor.matmul(
                    ot[64:128, 0:NT],
                    vt[b][0][:, h1 * HD:(h1 + 1) * HD],
                    pj1[:, 1, :], start=True, stop=False,
                    tile_position=(0, 64))
                nc.tensor.matmul(
                    ot[0:64, 0:NT],
                    vt[b][1][0:NT2, h0 * HD:(h0 + 1) * HD],
                    pj2[0:NT2, 0, :], start=False, stop=True)
                nc.tensor.matmul(
                    ot[64:128, 0:NT],
                    vt[b][1][0:NT2, h1 * HD:(h1 + 1) * HD],
                    pj2[0:NT2, 1, :], start=False, stop=True,
                    tile_position=(0, 64))
                # denominators: ones[128,64] lhsT, heads col-packed into
                # cols 256:256+NT of the same tile (partitions 0:64 = h0,
                # 64:128 = h1); accumulation groups in one PSUM bank must
                # stay sequential (interleaving open groups corrupts them)
                nc.tensor.matmul(
                    ot[0:64, 256:256 + NT], ones64[:, :],
                    pj1[:, 0, :], start=True, stop=False)
                nc.tensor.matmul(
                    ot[64:128, 256:256 + NT], ones64[:, :],
                    pj1[:, 1, :], start=True, stop=False,
                    tile_position=(0, 64))
                nc.tensor.matmul(
                    ot[0:64, 256:256 + NT], ones64[0:NT2, :],
                    pj2[0:NT2, 0, :], start=False, stop=True)
                nc.tensor.matmul(
                    ot[64:128, 256:256 + NT], ones64[0:NT2, :],
                    pj2[0:NT2, 1, :], start=False, stop=True,
                    tile_position=(0, 64))
                rc = rcp.tile([128, NT], F32, name="rc", tag="rc")
                nc.vector.reciprocal_approx_fast(
                    out=rc[:, :], in_=ot[:, 256:256 + NT])
                nc.vector.tensor_mul(
                    op_s[:, hp, b * NT:(b + 1) * NT],
                    ot[:, 0:NT], rc[:, :])

            # ---- projection over a token range (1-2 batches) ----
            def emit_proj(t0, t1):
                w = t1 - t0
                for c in range(KC):
                    acc = mm.tile([128, 512], F32, name="acc_p", tag="mm")
                    for kp in range(KC):
                        nc.tensor.matmul(
                            acc[:, 0:w],
                            pw_s[:, kp, c * 128:(c + 1) * 128],
                            op_s[:, kp, t0:t1],
                            start=(kp == 0),
                            stop=(kp == KC - 1) and not proj_bias_nonzero,
                        )
                    if proj_bias_nonzero:
                        nc.tensor.matmul(
                            acc[:, 0:w],
                            pb_s[0:1, c * 128:(c + 1) * 128],
                            ones_bfr[0:1, 0:w],
                            start=False, stop=True,
                        )
                    obt = obp.tile([128, 2 * NT], BF16, name="obt", tag="ob")
                    if c % 2 == 0:
                        nc.vector.tensor_copy(obt[:, 0:w], acc[:, 0:w])
                    else:
                        nc.scalar.copy(obt[:, 0:w], acc[:, 0:w])
                    nc.sync.dma_start(out[c * 128:(c + 1) * 128, t0:t1],
                                      obt[:, 0:w])

            # ---- batch-major schedule: v[b] -> items -> proj[b] ----
            items = [(b, hp) for b in range(BPC) for hp in range(KC)]
            pend = {}

            def run_stage_b(i):
                b, hp = items[i]
                stage_b(b, hp, *pend.pop(i))
                if hp == KC - 1:
                    # batches 0-5 project in pairs (394-wide rhs halves
                    # per-matmul overhead); 6 and 7 go solo so the last
                    # batch's projection isn't delayed into the tail
                    if b in (1, 3, 5):
                        emit_proj((b - 1) * NT, (b + 1) * NT)
                    elif b >= 6:
                        emit_proj(b * NT, (b + 1) * NT)

            for i, (b, hp) in enumerate(items):
                if hp == 0:
                    emit_v(b)
                pend[i] = stage_a(b, hp)
                if i >= SKEW:
                    run_stage_b(i - SKEW)
            for i in range(len(items) - SKEW, len(items)):
                run_stage_b(i)

    nc.compile()
    return nc


@functools.lru_cache(maxsize=4)
def _built(qkv_bias_nonzero: bool, proj_bias_nonzero: bool):
    return build(qkv_bias_nonzero, proj_bias_nonzero)


def prepare_inputs(x, qkv_w, q_bias, v_bias, rpb_table, proj_w, proj_b, rel_index):
    """Host-side prep: shard + transpose + fold scale + gather bias table."""
    x = np.asarray(x, dtype=np.float32)
    qkv_w = np.asarray(qkv_w, dtype=np.float32)
    q_bias = np.asarray(q_bias, dtype=np.float32)
    v_bias = np.asarray(v_bias, dtype=np.float32)
    rpb_table = np.asarray(rpb_table, dtype=np.float32)
    proj_w = np.asarray(proj_w, dtype=np.float32)
    proj_b = np.asarray(proj_b, dtype=np.float32)
    rel_index = np.asarray(rel_index)

    qw = qkv_w[0:DIM] * np.float32(SCALE)   # exact: SCALE is a power of two
    qkw_h = np.ascontiguousarray(
        np.concatenate([qw, qkv_w[DIM:2 * DIM]], axis=0).T).astype(
        ml_dtypes.bfloat16)                                      # [768, 1536]
    vw_h = np.ascontiguousarray(qkv_w[2 * DIM:3 * DIM].T).astype(
        ml_dtypes.bfloat16)                                      # [768, 768]
    pw_h = np.ascontiguousarray(proj_w.T).astype(ml_dtypes.bfloat16)

    # bias[i, j, h] -> exp -> ebT[h, j, i]
    bias = rpb_table[rel_index]                                  # (197,197,12)
    ebT = np.exp(bias.astype(np.float32)).transpose(2, 1, 0)     # (12, j, i)
    eb1_h = np.ascontiguousarray(
        ebT[:, 0:128, :].transpose(1, 0, 2).reshape(128, H * NT)
    ).astype(ml_dtypes.bfloat16)
    eb2_h = np.ascontiguousarray(
        ebT[:, 128:NT, :].transpose(1, 0, 2).reshape(NT2, H * NT)
    ).astype(ml_dtypes.bfloat16)

    qkv_bias_nonzero = bool(q_bias.any() or v_bias.any())
    proj_bias_nonzero = bool(proj_b.any())

    in_maps = []
    for i in range(NCORES):
        xs = x[i * BPC:(i + 1) * BPC].reshape(TOK, DIM)
        m = {
            "xt": np.ascontiguousarray(xs.T).astype(ml_dtypes.bfloat16),
            "qkw": qkw_h, "vw": vw_h, "pw": pw_h,
            "eb1": eb1_h, "eb2": eb2_h,
        }
        if qkv_bias_nonzero:
            m["qkb"] = np.ascontiguousarray(
                np.concatenate([q_bias * np.float32(SCALE),
                                np.zeros_like(q_bias)])[None, :],
                dtype=np.float32).astype(ml_dtypes.bfloat16)
            m["vb"] = np.ascontiguousarray(
                v_bias[None, :]).astype(ml_dtypes.bfloat16)
        if proj_bias_nonzero:
            m["pb"] = np.ascontiguousarray(
                proj_b[None, :], dtype=np.float32).astype(ml_dtypes.bfloat16)
        in_maps.append(m)
    return in_maps, qkv_bias_nonzero, proj_bias_nonzero


def kernel(x, qkv_w, q_bias, v_bias, rpb_table, proj_w, proj_b, rel_index):
    in_maps, qb_nz, pb_nz = prepare_inputs(
        x, qkv_w, q_bias, v_bias, rpb_table, proj_w, proj_b, rel_index)
    nc = _built(qb_nz, pb_nz)
    res = run_bass_kernel_spmd(nc, in_maps, core_ids=list(range(NCORES)))
    outs = []
    for i in range(NCORES):
        ofm = res.results[i]["out"]                  # [768, 1576]
        outs.append(ofm.T.reshape(BPC, NT, DIM))
    return np.concatenate(outs, axis=0).astype(np.float32)


# revision 24
# speedup vs baseline: 1.1548x; 1.0061x over previous
"""Trainium2 Bass kernel for windowed multi-head attention (nn_AttentionWindow).

Reference computation (B=64, N=197, DIM=768, H=12, HD=64):
    qkv  = x @ qkv_w.T + [q_bias, 0, v_bias]
    q, k, v = split(qkv);  q *= HD**-0.5
    attn = softmax(q @ k.T + rpb_table[rel_index])
    out  = (attn @ v) @ proj_w.T + proj_b

Sharding: data-parallel over batch across 8 NeuronCores (8 batches/core).

Per-core design (bf16 matmuls on TensorE, fp32 PSUM accumulation):
  - x pre-transposed on host to xT [768, 1576] bf16 (feature-major),
    resident in SBUF. q,k computed feature-major into one resident
    [128, 12, 1576] tile, tiled over 512-token slices (batch-agnostic).
  - v: token-major per batch ([tokens, channels], 128+69 row chunks) so
    the attention contraction has tokens on partitions; stored bf16.
  - Scores transposed: S^T[j,i] = k_h[:,j]^T q_h, per (head-pair,
    batch) item. The two heads of a pair use opposite 64-row halves of
    the PE array (row-packing -> concurrent matmuls). Softmax WITHOUT
    max subtraction (scores are O(1): q pre-scaled by 1/8): exp on
    ScalarE (both heads' score chunks merged into one strided
    activation) -> P^T bf16, then bias multiply on VectorE against the
    precomputed exp(bias) table: softmax(S+B) = expS*expB / sums.
  - O^T[d,i] = sum_j v[j,d] P^T[j,i]: head pair col-packed via
    tile_position (0,0)/(0,64); softmax denominators via ones[128,64]
    matmuls, ALSO col-packed by head into the same PSUM tile as O^T
    (cols 256:453, accumulation groups kept sequential) -> one
    reciprocal_approx_fast + one merged normalization multiply per
    item on VectorE.
  - Schedule is batch-major: qk chunks first (emission follows DMA
    arrival: q chunks over the first token half, then the rest), then
    per batch v[b] -> 6 attention items -> proj[b] (batches paired
    394-wide where it doesn't lengthen the tail); the v/proj matmul
    streams fill TensorE during softmax latencies and the bf16 output
    DMA overlaps.
  - PSUM->SBUF copies alternate ScalarE/VectorE; all input DMAs go on
    the sync hardware-DGE queue (engine-issued DMAs take slow software
    queues).
"""
import sys
import functools

sys.path.insert(0, "/opt/trn_rl_repo")

import numpy as np
import ml_dtypes

import concourse.bass as bass  # noqa: E402
import concourse.bacc as bacc  # noqa: E402
import concourse.mybir as mybir  # noqa: E402
from concourse.tile import TileContext  # noqa: E402
from concourse.bass_utils import run_bass_kernel_spmd  # noqa: E402

F32 = mybir.dt.float32
BF16 = mybir.dt.bfloat16

NCORES = 8
B, NT, DIM = 64, 197, 768
H, HD = 12, 64
SCALE = HD ** -0.5  # 0.125, exact power of two -> folded into q weights
BPC = B // NCORES   # 8 batches per core
TOK = BPC * NT      # 1576 tokens per core
KC = DIM // 128     # 6
NT2 = NT - 128      # 69 (second token chunk)
SKEW = 2            # attention software-pipeline depth (items)
# token slices for the token-parallel qk matmul (PSUM bank limit 512);
# the tail is split 276+276 rather than 512+40 so no matmul is so short
# that its weight load dominates
SLICES = [(0, 512), (512, 1024), (1024, 1300), (1300, TOK)]


def build(qkv_bias_nonzero: bool, proj_bias_nonzero: bool):
    nc = bacc.Bacc("TRN2", target_bir_lowering=False, debug=False)

    xt = nc.dram_tensor("xt", [DIM, TOK], BF16, kind="ExternalInput")
    qkw = nc.dram_tensor("qkw", [DIM, 2 * DIM], BF16, kind="ExternalInput")
    vw = nc.dram_tensor("vw", [DIM, DIM], BF16, kind="ExternalInput")
    pw = nc.dram_tensor("pw", [DIM, DIM], BF16, kind="ExternalInput")
    eb1 = nc.dram_tensor("eb1", [128, H * NT], BF16, kind="ExternalInput")
    eb2 = nc.dram_tensor("eb2", [NT2, H * NT], BF16, kind="ExternalInput")
    out = nc.dram_tensor("out", [DIM, TOK], BF16, kind="ExternalOutput")
    if qkv_bias_nonzero:
        qkb = nc.dram_tensor("qkb", [1, 2 * DIM], BF16, kind="ExternalInput")
        vb = nc.dram_tensor("vb", [1, DIM], BF16, kind="ExternalInput")
    if proj_bias_nonzero:
        pb = nc.dram_tensor("pb", [1, DIM], BF16, kind="ExternalInput")

    with TileContext(nc) as tc:
        with (
            tc.tile_pool(name="const", bufs=1) as constp,
            tc.tile_pool(name="vp", bufs=8) as vp,
            tc.tile_pool(name="pp", bufs=12) as pp,
            tc.tile_pool(name="rcp", bufs=3) as rcp,
            tc.tile_pool(name="obp", bufs=4) as obp,
            tc.tile_pool(name="mm", bufs=2, space="PSUM") as mm,
            tc.tile_pool(name="ots", bufs=2, space="PSUM") as ots,
            tc.tile_pool(name="sta", bufs=2, space="PSUM") as sta,
        ):
            # ---- resident constants & activations ----
            # DMA order matters: qk needs xb+qkw first; vw next (v of
            # batch 0 runs right after qk); eb tables before the first
            # bias-multiply; pw last (proj is latest consumer).
            xb_s = constp.tile([128, KC, TOK], BF16, name="xb_s")
            qkw_s = constp.tile([128, KC, 2 * DIM], BF16, name="qkw_s")
            vw_s = constp.tile([128, KC, DIM], BF16, name="vw_s")
            pw_s = constp.tile([128, KC, DIM], BF16, name="pw_s")
            # all input DMAs on the sync hardware-DGE queue (engine-issued
            # DMAs fall back to slow software-dynamic queues); xb/qkw
            # chunks interleaved so qk compute starts feeding ASAP
            for kc in range(KC):
                nc.sync.dma_start(xb_s[:, kc, 0:1024],
                                  xt[kc * 128:(kc + 1) * 128, 0:1024])
                nc.sync.dma_start(qkw_s[:, kc, 0:DIM],
                                  qkw[kc * 128:(kc + 1) * 128, 0:DIM])
            for kc in range(KC):
                nc.sync.dma_start(xb_s[:, kc, 1024:TOK],
                                  xt[kc * 128:(kc + 1) * 128, 1024:TOK])
            for kc in range(KC):
                nc.sync.dma_start(qkw_s[:, kc, DIM:2 * DIM],
                                  qkw[kc * 128:(kc + 1) * 128, DIM:2 * DIM])
            for kc in range(KC):
                nc.sync.dma_start(vw_s[:, kc, :], vw[kc * 128:(kc + 1) * 128, :])
            eb1_s = constp.tile([128, H * NT], BF16, name="eb1_s")
            eb2_s = constp.tile([NT2, H * NT], BF16, name="eb2_s")
            nc.sync.dma_start(eb1_s[:, :], eb1[:, :])
            nc.sync.dma_start(eb2_s[:, :], eb2[:, :])
            for kc in range(KC):
                nc.sync.dma_start(pw_s[:, kc, :], pw[kc * 128:(kc + 1) * 128, :])
            # ones [128, 64] lhsT for the col-packed denominator matmuls
            ones64 = constp.tile([128, 64], BF16, name="ones64")
            nc.gpsimd.memset(ones64[:, :], 1.0)
            # big resident activations: q,k and proj-rhs (bf16)
            qk_s = constp.tile([128, 2 * KC, TOK], BF16, name="qk_s")
            op_s = constp.tile([128, KC, TOK], BF16, name="op_s")
            if qkv_bias_nonzero:
                qkb_s = constp.tile([1, 2 * DIM], BF16, name="qkb_s")
                vb_s = constp.tile([1, DIM], BF16, name="vb_s")
                nc.sync.dma_start(qkb_s[:, :], qkb[:, :])
                nc.sync.dma_start(vb_s[:, :], vb[:, :])
            if proj_bias_nonzero:
                pb_s = constp.tile([1, DIM], BF16, name="pb_s")
                nc.sync.dma_start(pb_s[:, :], pb[:, :])
            if qkv_bias_nonzero or proj_bias_nonzero:
                ones_bfr = constp.tile([1, 512], BF16, name="ones_bfr")
                nc.gpsimd.memset(ones_bfr[:, :], 1.0)

            # ---- q,k feature-major: 12 channel-chunks x token slices ----
            # chunk order follows DMA arrival: q chunks over the first
            # token half, then q over the second half, then k chunks
            qk_order = ([(c, s) for c in range(KC) for s in (0, 1)]
                        + [(c, s) for c in range(KC) for s in (2, 3)]
                        + [(c, s) for c in range(KC, 2 * KC)
                           for s in range(4)])
            for c, si in qk_order:
                    t0, t1 = SLICES[si]
                    acc = mm.tile([128, 512], F32, name="acc_qk", tag="mm")
                    w = t1 - t0
                    for kc in range(KC):
                        nc.tensor.matmul(
                            acc[:, 0:w],
                            qkw_s[:, kc, c * 128:(c + 1) * 128],
                            xb_s[:, kc, t0:t1],
                            start=(kc == 0),
                            stop=(kc == KC - 1) and not qkv_bias_nonzero,
                        )
                    if qkv_bias_nonzero:
                        nc.tensor.matmul(
                            acc[:, 0:w],
                            qkb_s[0:1, c * 128:(c + 1) * 128],
                            ones_bfr[0:1, 0:w],
                            start=False, stop=True,
                        )
                    # PSUM->SBUF: alternate scalar/vector (both idle here)
                    if (c * len(SLICES) + si) % 2 == 0:
                        nc.scalar.copy(qk_s[:, c, t0:t1], acc[:, 0:w])
                    else:
                        nc.vector.tensor_copy(qk_s[:, c, t0:t1], acc[:, 0:w])

            # ---- v token-major for one batch: [(128|69) tok, 768 ch] ----
            vt = [[None, None] for _ in range(BPC)]

            def emit_v(b):
                for tch in range(2):
                    toff = b * NT + tch * 128
                    tlen = 128 if tch == 0 else NT2
                    t = vp.tile([128, DIM], BF16, name="v_t", tag="v")
                    for half in range(2):
                        n0, n1 = half * 384, (half + 1) * 384
                        acc = mm.tile([128, 384], F32, name="acc_v", tag="mm")
                        for kc in range(KC):
                            nc.tensor.matmul(
                                acc[0:tlen, :],
                                xb_s[:, kc, toff:toff + tlen],
                                vw_s[:, kc, n0:n1],
                                start=(kc == 0),
                                stop=(kc == KC - 1) and not qkv_bias_nonzero,
                            )
                        if qkv_bias_nonzero:
                            nc.tensor.matmul(
                                acc[0:tlen, :],
                                ones_bfr[0:1, 0:tlen],
                                vb_s[0:1, n0:n1],
                                start=False, stop=True,
                            )
                        if half == 0:
                            nc.scalar.copy(t[0:tlen, n0:n1], acc[0:tlen, :])
                        else:
                            nc.vector.tensor_copy(t[0:tlen, n0:n1],
                                                  acc[0:tlen, :])
                    vt[b][tch] = t

            # ---- attention stages (per (batch, head-pair) item) ----
            def stage_a(b, hp):
                """Scores for heads 2hp,2hp+1 (array row-packed), merged
                exp + bias-mult (j1 on vector, j2 on gpsimd) -> P^T."""
                h0 = 2 * hp
                st = sta.tile([128, 1024], F32, name="st", tag="sta")
                q0 = qk_s[0:64, hp, b * NT:(b + 1) * NT]
                q1 = qk_s[64:128, hp, b * NT:(b + 1) * NT]
                # j1 chunks (K rows 0:64 then 64:128 -> packed), then j2
                nc.tensor.matmul(st[:, 0:NT],
                                 qk_s[0:64, KC + hp, b * NT:b * NT + 128],
                                 q0, start=True, stop=True)
                nc.tensor.matmul(st[:, 512:512 + NT],
                                 qk_s[64:128, KC + hp, b * NT:b * NT + 128],
                                 q1, start=True, stop=True)
                nc.tensor.matmul(st[0:NT2, 256:256 + NT],
                                 qk_s[0:64, KC + hp, b * NT + 128:(b + 1) * NT],
                                 q0, start=True, stop=True)
                nc.tensor.matmul(st[0:NT2, 768:768 + NT],
                                 qk_s[64:128, KC + hp, b * NT + 128:(b + 1) * NT],
                                 q1, start=True, stop=True)
                # merged exp over both heads (strided free AP), bf16 out
                pj1 = pp.tile([128, 2, NT], BF16, name="pj1", tag="p")
                nc.scalar.activation(
                    pj1[:, :, :], st[:, 0:1024].rearrange("p (h x) -> p h x", h=2)[:, :, 0:NT],
                    mybir.ActivationFunctionType.Exp)
                pj2 = pp.tile([128, 2, NT], BF16, name="pj2", tag="p")
                nc.scalar.activation(
                    pj2[0:NT2, :, :],
                    st[0:NT2, 0:1024].rearrange("p (h x) -> p h x", h=2)[:, :, 256:256 + NT],
                    mybir.ActivationFunctionType.Exp)
                # bias multiply (heads adjacent in eb tables)
                nc.vector.tensor_mul(
                    pj1[:, :, :].rearrange("p h x -> p (h x)"),
                    pj1[:, :, :].rearrange("p h x -> p (h x)"),
                    eb1_s[:, h0 * NT:(h0 + 2) * NT])
                nc.vector.tensor_mul(
                    pj2[0:NT2, :, :].rearrange("p h x -> p (h x)"),
                    pj2[0:NT2, :, :].rearrange("p h x -> p (h x)"),
                    eb2_s[:, h0 * NT:(h0 + 2) * NT])
                return pj1, pj2

            def stage_b(b, hp, pj1, pj2):
                """O^T and denominators, both head-pair col-packed into
                one PSUM tile; one recip + one merged normalize."""
                h0, h1 = 2 * hp, 2 * hp + 1
                ot = ots.tile([128, 512], F32, name="ot", tag="ot")
                nc.tensor.matmul(
                    ot[0:64, 0:NT],
                    vt[b][0][:, h0 * HD:(h0 + 1) * HD],
                    pj1[:, 0, :], start=True, stop=False)
                nc.tensor.matmul(
                    ot[64:128, 0:NT],
                    vt[b][0][:, h1 * HD:(h1 + 1) * HD],
                    pj1[:, 1, :], start=True, stop=False,
                    tile_position=(0, 64))
                nc.tensor.matmul(
                    ot[0:64, 0:NT],
                    vt[b][1][0:NT2, h0 * HD:(h0 + 1) * HD],
                    pj2[0:NT2, 0, :], start=False, stop=True)
                nc.tensor.matmul(
                    ot[64:128, 0:NT],
                    vt[b][1][0:NT2, h1 * HD:(h1 + 1) * HD],
                    pj2[0:NT2, 1, :], start=False, stop=True,
                    tile_position=(0, 64))
                # denominators: ones[128,64] lhsT, heads col-packed into
                # cols 256:256+NT of the same tile (partitions 0:64 = h0,
                # 64:128 = h1); accumulation groups in one PSUM bank must
                # stay sequential (interleaving open groups corrupts them)
                nc.tensor.matmul(
                    ot[0:64, 256:256 + NT], ones64[:, :],
                    pj1[:, 0, :], start=True, stop=False)
                nc.tensor.matmul(
                    ot[64:128, 256:256 + NT], ones64[:, :],
                    pj1[:, 1, :], start=True, stop=False,
                    tile_position=(0, 64))
                nc.tensor.matmul(
                    ot[0:64, 256:256 + NT], ones64[0:NT2, :],
                    pj2[0:NT2, 0, :], start=False, stop=True)
                nc.tensor.matmul(
                    ot[64:128, 256:256 + NT], ones64[0:NT2, :],
                    pj2[0:NT2, 1, :], start=False, stop=True,
                    tile_position=(0, 64))
                rc = rcp.tile([128, NT], F32, name="rc", tag="rc")
                nc.vector.reciprocal_approx_fast(
                    out=rc[:, :], in_=ot[:, 256:256 + NT])
                nc.vector.tensor_mul(
                    op_s[:, hp, b * NT:(b + 1) * NT],
                    ot[:, 0:NT], rc[:, :])

            # ---- projection over a token range (1-2 batches) ----
            def emit_proj(t0, t1):
                w = t1 - t0
                for c in range(KC):
                    acc = mm.tile([128, 512], F32, name="acc_p", tag="mm")
                    for kp in range(KC):
                        nc.tensor.matmul(
                            acc[:, 0:w],
                            pw_s[:, kp, c * 128:(c + 1) * 128],
                            op_s[:, kp, t0:t1],
                            start=(kp == 0),
                            stop=(kp == KC - 1) and not proj_bias_nonzero,
                        )
                    if proj_bias_nonzero:
                        nc.tensor.matmul(
                            acc[:, 0:w],
                            pb_s[0:1, c * 128:(c + 1) * 128],
                            ones_bfr[0:1, 0:w],
                            start=False, stop=True,
                        )
                    obt = obp.tile([128, 2 * NT], BF16, name="obt", tag="ob")
                    if c % 2 == 0:
                        nc.vector.tensor_copy(obt[:, 0:w], acc[:, 0:w])
                    else:
                        nc.scalar.copy(obt[:, 0:w], acc[:, 0:w])
                    nc.sync.dma_start(out[c * 128:(c + 1) * 128, t0:t1],
                                      obt[:, 0:w])

            # ---- batch-major schedule: v[b] -> items -> proj[b] ----
            items = [(b, hp) for b in range(BPC) for hp in range(KC)]
            pend = {}

            def run_stage_b(i):
                b, hp = items[i]
                stage_b(b, hp, *pend.pop(i))
                if hp == KC - 1:
                    # batches 0-5 project in pairs (394-wide rhs halves
                    # per-matmul overhead); 6 and 7 go solo so the last
                    # batch's projection isn't delayed into the tail
                    if b in (1, 3, 5):
                        emit_proj((b - 1) * NT, (b + 1) * NT)
                    elif b >= 6:
                        emit_proj(b * NT, (b + 1) * NT)

            for i, (b, hp) in enumerate(items):
                if hp == 0:
                    emit_v(b)
                pend[i] = stage_a(b, hp)
                if i >= SKEW:
                    run_stage_b(i - SKEW)
            for i in range(len(items) - SKEW, len(items)):
                run_stage_b(i)

    nc.compile()
    return nc


@functools.lru_cache(maxsize=4)
def _built(qkv_bias_nonzero: bool, proj_bias_nonzero: bool):
    return build(qkv_bias_nonzero, proj_bias_nonzero)


def prepare_inputs(x, qkv_w, q_bias, v_bias, rpb_table, proj_w, proj_b, rel_index):
    """Host-side prep: shard + transpose + fold scale + gather bias table."""
    x = np.asarray(x, dtype=np.float32)
    qkv_w = np.asarray(qkv_w, dtype=np.float32)
    q_bias = np.asarray(q_bias, dtype=np.float32)
    v_bias = np.asarray(v_bias, dtype=np.float32)
    rpb_table = np.asarray(rpb_table, dtype=np.float32)
    proj_w = np.asarray(proj_w, dtype=np.float32)
    proj_b = np.asarray(proj_b, dtype=np.float32)
    rel_index = np.asarray(rel_index)

    qw = qkv_w[0:DIM] * np.float32(SCALE)   # exact: SCALE is a power of two
    qkw_h = np.ascontiguousarray(
        np.concatenate([qw, qkv_w[DIM:2 * DIM]], axis=0).T).astype(
        ml_dtypes.bfloat16)                                      # [768, 1536]
    vw_h = np.ascontiguousarray(qkv_w[2 * DIM:3 * DIM].T).astype(
        ml_dtypes.bfloat16)                                      # [768, 768]
    pw_h = np.ascontiguousarray(proj_w.T).astype(ml_dtypes.bfloat16)

    # bias[i, j, h] -> exp -> ebT[h, j, i]
    bias = rpb_table[rel_index]                                  # (197,197,12)
    ebT = np.exp(bias.astype(np.float32)).transpose(2, 1, 0)     # (12, j, i)
    eb1_h = np.ascontiguousarray(
        ebT[:, 0:128, :].transpose(1, 0, 2).reshape(128, H * NT)
    ).astype(ml_dtypes.bfloat16)
    eb2_h = np.ascontiguousarray(
        ebT[:, 128:NT, :].transpose(1, 0, 2).reshape(NT2, H * NT)
    ).astype(ml_dtypes.bfloat16)

    qkv_bias_nonzero = bool(q_bias.any() or v_bias.any())
    proj_bias_nonzero = bool(proj_b.any())

    in_maps = []
    for i in range(NCORES):
        xs = x[i * BPC:(i + 1) * BPC].reshape(TOK, DIM)
        m = {
            "xt": np.ascontiguousarray(xs.T).astype(ml_dtypes.bfloat16),
            "qkw": qkw_h, "vw": vw_h, "pw": pw_h,
            "eb1": eb1_h, "eb2": eb2_h,
        }
        if qkv_bias_nonzero:
            m["qkb"] = np.ascontiguousarray(
                np.concatenate([q_bias * np.float32(SCALE),
                                np.zeros_like(q_bias)])[None, :],
                dtype=np.float32).astype(ml_dtypes.bfloat16)
            m["vb"] = np.ascontiguousarray(
                v_bias[None, :]).astype(ml_dtypes.bfloat16)
        if proj_bias_nonzero:
            m["pb"] = np.ascontiguousarray(
                proj_b[None, :], dtype=np.float32).astype(ml_dtypes.bfloat16)
        in_maps.append(m)
    return in_maps, qkv_bias_nonzero, proj_bias_nonzero


def kernel(x, qkv_w, q_bias, v_bias, rpb_table, proj_w, proj_b, rel_index):
    in_maps, qb_nz, pb_nz = prepare_inputs(
        x, qkv_w, q_bias, v_bias, rpb_table, proj_w, proj_b, rel_index)
    nc = _built(qb_nz, pb_nz)
    res = run_bass_kernel_spmd(nc, in_maps, core_ids=list(range(NCORES)))
    outs = []
    for i in range(NCORES):
        ofm = res.results[i]["out"]                  # [768, 1576]
        outs.append(ofm.T.reshape(BPC, NT, DIM))
    return np.concatenate(outs, axis=0).astype(np.float32)


# revision 27
# speedup vs baseline: 1.1583x; 1.0030x over previous
"""Trainium2 Bass kernel for windowed multi-head attention (nn_AttentionWindow).

Reference computation (B=64, N=197, DIM=768, H=12, HD=64):
    qkv  = x @ qkv_w.T + [q_bias, 0, v_bias]
    q, k, v = split(qkv);  q *= HD**-0.5
    attn = softmax(q @ k.T + rpb_table[rel_index])
    out  = (attn @ v) @ proj_w.T + proj_b

Sharding: data-parallel over batch across 8 NeuronCores (8 batches/core).

Per-core design (bf16 matmuls on TensorE, fp32 PSUM accumulation):
  - x pre-transposed on host to xT [768, 1576] bf16 (feature-major),
    resident in SBUF. q,k computed feature-major into one resident
    [128, 12, 1576] tile, tiled over 512-token slices (batch-agnostic).
  - v: token-major per batch ([tokens, channels], 128+69 row chunks) so
    the attention contraction has tokens on partitions; stored bf16.
  - Scores transposed: S^T[j,i] = k_h[:,j]^T q_h, per (head-pair,
    batch) item. The two heads of a pair use opposite 64-row halves of
    the PE array (row-packing -> concurrent matmuls). Softmax WITHOUT
    max subtraction (scores are O(1): q pre-scaled by 1/8): exp on
    ScalarE (both heads' score chunks merged into one strided
    activation) -> P^T bf16, then bias multiply on VectorE against the
    precomputed exp(bias) table: softmax(S+B) = expS*expB / sums.
  - O^T[d,i] = sum_j v[j,d] P^T[j,i]: head pair col-packed via
    tile_position (0,0)/(0,64); softmax denominators via ones[128,64]
    matmuls, ALSO col-packed by head into the same PSUM tile as O^T
    (cols 256:453, accumulation groups kept sequential) -> one
    reciprocal_approx_fast + one merged normalization multiply per
    item on VectorE.
  - Schedule is batch-major: qk chunks first (emission follows DMA
    arrival: q chunks over the first token half, then the rest), then
    per batch v[b] -> 6 attention items -> proj[b] (batches paired
    394-wide where it doesn't lengthen the tail); the v/proj matmul
    streams fill TensorE during softmax latencies and the bf16 output
    DMA overlaps.
  - PSUM->SBUF copies alternate ScalarE/VectorE; all input DMAs go on
    the sync hardware-DGE queue (engine-issued DMAs take slow software
    queues).
"""
import sys
import functools

sys.path.insert(0, "/opt/trn_rl_repo")

import numpy as np
import ml_dtypes

import concourse.bass as bass  # noqa: E402
import concourse.bacc as bacc  # noqa: E402
import concourse.mybir as mybir  # noqa: E402
from concourse.tile import TileContext  # noqa: E402
from concourse.bass_utils import run_bass_kernel_spmd  # noqa: E402

F32 = mybir.dt.float32
BF16 = mybir.dt.bfloat16

NCORES = 8
B, NT, DIM = 64, 197, 768
H, HD = 12, 64
SCALE = HD ** -0.5  # 0.125, exact power of two -> folded into q weights
BPC = B // NCORES   # 8 batches per core
TOK = BPC * NT      # 1576 tokens per core
KC = DIM // 128     # 6
NT2 = NT - 128      # 69 (second token chunk)
SKEW = 3            # attention software-pipeline depth (items)
# token slices for the token-parallel qk matmul (PSUM bank limit 512);
# the tail is split 276+276 rather than 512+40 so no matmul is so short
# that its weight load dominates
SLICES = [(0, 512), (512, 1024), (1024, 1300), (1300, TOK)]


def build(qkv_bias_nonzero: bool, proj_bias_nonzero: bool):
    nc = bacc.Bacc("TRN2", target_bir_lowering=False, debug=False)

    xt = nc.dram_tensor("xt", [DIM, TOK], BF16, kind="ExternalInput")
    qkw = nc.dram_tensor("qkw", [DIM, 2 * DIM], BF16, kind="ExternalInput")
    vw = nc.dram_tensor("vw", [DIM, DIM], BF16, kind="ExternalInput")
    pw = nc.dram_tensor("pw", [DIM, DIM], BF16, kind="ExternalInput")
    eb1 = nc.dram_tensor("eb1", [128, H * NT], BF16, kind="ExternalInput")
    eb2 = nc.dram_tensor("eb2", [NT2, H * NT], BF16, kind="ExternalInput")
    out = nc.dram_tensor("out", [DIM, TOK], BF16, kind="ExternalOutput")
    if qkv_bias_nonzero:
        qkb = nc.dram_tensor("qkb", [1, 2 * DIM], BF16, kind="ExternalInput")
        vb = nc.dram_tensor("vb", [1, DIM], BF16, kind="ExternalInput")
    if proj_bias_nonzero:
        pb = nc.dram_tensor("pb", [1, DIM], BF16, kind="ExternalInput")

    with TileContext(nc) as tc:
        with (
            tc.tile_pool(name="const", bufs=1) as constp,
            tc.tile_pool(name="vp", bufs=8) as vp,
            tc.tile_pool(name="pp", bufs=12) as pp,
            tc.tile_pool(name="rcp", bufs=3) as rcp,
            tc.tile_pool(name="obp", bufs=4) as obp,
            tc.tile_pool(name="mm", bufs=2, space="PSUM") as mm,
            tc.tile_pool(name="sta", bufs=3, space="PSUM") as sta,
        ):
            # ---- resident constants & activations ----
            # DMA order matters: qk needs xb+qkw first; vw next (v of
            # batch 0 runs right after qk); eb tables before the first
            # bias-multiply; pw last (proj is latest consumer).
            xb_s = constp.tile([128, KC, TOK], BF16, name="xb_s")
            qkw_s = constp.tile([128, KC, 2 * DIM], BF16, name="qkw_s")
            vw_s = constp.tile([128, KC, DIM], BF16, name="vw_s")
            pw_s = constp.tile([128, KC, DIM], BF16, name="pw_s")
            # all input DMAs on the sync hardware-DGE queue (engine-issued
            # DMAs fall back to slow software-dynamic queues); xb/qkw
            # chunks interleaved so qk compute starts feeding ASAP
            for kc in range(KC):
                nc.sync.dma_start(xb_s[:, kc, 0:1024],
                                  xt[kc * 128:(kc + 1) * 128, 0:1024])
                nc.sync.dma_start(qkw_s[:, kc, 0:DIM],
                                  qkw[kc * 128:(kc + 1) * 128, 0:DIM])
            for kc in range(KC):
                nc.sync.dma_start(xb_s[:, kc, 1024:TOK],
                                  xt[kc * 128:(kc + 1) * 128, 1024:TOK])
            for kc in range(KC):
                nc.sync.dma_start(qkw_s[:, kc, DIM:2 * DIM],
                                  qkw[kc * 128:(kc + 1) * 128, DIM:2 * DIM])
            for kc in range(KC):
                nc.sync.dma_start(vw_s[:, kc, :], vw[kc * 128:(kc + 1) * 128, :])
            eb1_s = constp.tile([128, H * NT], BF16, name="eb1_s")
            eb2_s = constp.tile([NT2, H * NT], BF16, name="eb2_s")
            nc.sync.dma_start(eb1_s[:, :], eb1[:, :])
            nc.sync.dma_start(eb2_s[:, :], eb2[:, :])
            for kc in range(KC):
                nc.sync.dma_start(pw_s[:, kc, :], pw[kc * 128:(kc + 1) * 128, :])
            # ones [128, 64] lhsT for the col-packed denominator matmuls
            ones64 = constp.tile([128, 64], BF16, name="ones64")
            nc.gpsimd.memset(ones64[:, :], 1.0)
            # big resident activations: q,k and proj-rhs (bf16)
            qk_s = constp.tile([128, 2 * KC, TOK], BF16, name="qk_s")
            op_s = constp.tile([128, KC, TOK], BF16, name="op_s")
            if qkv_bias_nonzero:
                qkb_s = constp.tile([1, 2 * DIM], BF16, name="qkb_s")
                vb_s = constp.tile([1, DIM], BF16, name="vb_s")
                nc.sync.dma_start(qkb_s[:, :], qkb[:, :])
                nc.sync.dma_start(vb_s[:, :], vb[:, :])
            if proj_bias_nonzero:
                pb_s = constp.tile([1, DIM], BF16, name="pb_s")
                nc.sync.dma_start(pb_s[:, :], pb[:, :])
            if qkv_bias_nonzero or proj_bias_nonzero:
                ones_bfr = constp.tile([1, 512], BF16, name="ones_bfr")
                nc.gpsimd.memset(ones_bfr[:, :], 1.0)

            # ---- q,k feature-major: 12 channel-chunks x token slices ----
            # chunk order follows DMA arrival: q chunks over the first
            # token half, then q over the second half, then k chunks
            qk_order = ([(c, s) for c in range(KC) for s in (0, 1)]
                        + [(c, s) for c in range(KC) for s in (2, 3)]
                        + [(c, s) for c in range(KC, 2 * KC)
                           for s in range(4)])
            for c, si in qk_order:
                    t0, t1 = SLICES[si]
                    acc = mm.tile([128, 512], F32, name="acc_qk", tag="mm")
                    w = t1 - t0
                    for kc in range(KC):
                        nc.tensor.matmul(
                            acc[:, 0:w],
                            qkw_s[:, kc, c * 128:(c + 1) * 128],
                            xb_s[:, kc, t0:t1],
                            start=(kc == 0),
                            stop=(kc == KC - 1) and not qkv_bias_nonzero,
                        )
                    if qkv_bias_nonzero:
                        nc.tensor.matmul(
                            acc[:, 0:w],
                            qkb_s[0:1, c * 128:(c + 1) * 128],
                            ones_bfr[0:1, 0:w],
                            start=False, stop=True,
                        )
                    # PSUM->SBUF: alternate scalar/vector (both idle here)
                    if (c * len(SLICES) + si) % 2 == 0:
                        nc.scalar.copy(qk_s[:, c, t0:t1], acc[:, 0:w])
                    else:
                        nc.vector.tensor_copy(qk_s[:, c, t0:t1], acc[:, 0:w])

            # ---- v token-major for one batch: [(128|69) tok, 768 ch] ----
            vt = [[None, None] for _ in range(BPC)]

            def emit_v(b):
                for tch in range(2):
                    toff = b * NT + tch * 128
                    tlen = 128 if tch == 0 else NT2
                    t = vp.tile([128, DIM], BF16, name="v_t", tag="v")
                    for half in range(2):
                        n0, n1 = half * 384, (half + 1) * 384
                        acc = mm.tile([128, 384], F32, name="acc_v", tag="mm")
                        for kc in range(KC):
                            nc.tensor.matmul(
                                acc[0:tlen, :],
                                xb_s[:, kc, toff:toff + tlen],
                                vw_s[:, kc, n0:n1],
                                start=(kc == 0),
                                stop=(kc == KC - 1) and not qkv_bias_nonzero,
                            )
                        if qkv_bias_nonzero:
                            nc.tensor.matmul(
                                acc[0:tlen, :],
                                ones_bfr[0:1, 0:tlen],
                                vb_s[0:1, n0:n1],
                                start=False, stop=True,
                            )
                        if half == 0:
                            nc.scalar.copy(t[0:tlen, n0:n1], acc[0:tlen, :])
                        else:
                            nc.vector.tensor_copy(t[0:tlen, n0:n1],
                                                  acc[0:tlen, :])
                    vt[b][tch] = t

            # ---- attention stages (per (batch, head-pair) item) ----
            def stage_a(b, hp):
                """Scores for heads 2hp,2hp+1 (array row-packed), merged
                exp + bias-mult (j1 on vector, j2 on gpsimd) -> P^T."""
                h0 = 2 * hp
                st = sta.tile([128, 1024], F32, name="st", tag="sta")
                q0 = qk_s[0:64, hp, b * NT:(b + 1) * NT]
                q1 = qk_s[64:128, hp, b * NT:(b + 1) * NT]
                # j1 chunks (K rows 0:64 then 64:128 -> packed), then j2
                nc.tensor.matmul(st[:, 0:NT],
                                 qk_s[0:64, KC + hp, b * NT:b * NT + 128],
                                 q0, start=True, stop=True)
                nc.tensor.matmul(st[:, 512:512 + NT],
                                 qk_s[64:128, KC + hp, b * NT:b * NT + 128],
                                 q1, start=True, stop=True)
                nc.tensor.matmul(st[0:NT2, 256:256 + NT],
                                 qk_s[0:64, KC + hp, b * NT + 128:(b + 1) * NT],
                                 q0, start=True, stop=True)
                nc.tensor.matmul(st[0:NT2, 768:768 + NT],
                                 qk_s[64:128, KC + hp, b * NT + 128:(b + 1) * NT],
                                 q1, start=True, stop=True)
                # merged exp over both heads (strided free AP), bf16 out
                pj1 = pp.tile([128, 2, NT], BF16, name="pj1", tag="p")
                nc.scalar.activation(
                    pj1[:, :, :], st[:, 0:1024].rearrange("p (h x) -> p h x", h=2)[:, :, 0:NT],
                    mybir.ActivationFunctionType.Exp)
                pj2 = pp.tile([128, 2, NT], BF16, name="pj2", tag="p")
                nc.scalar.activation(
                    pj2[0:NT2, :, :],
                    st[0:NT2, 0:1024].rearrange("p (h x) -> p h x", h=2)[:, :, 256:256 + NT],
                    mybir.ActivationFunctionType.Exp)
                # bias multiply (heads adjacent in eb tables)
                nc.vector.tensor_mul(
                    pj1[:, :, :].rearrange("p h x -> p (h x)"),
                    pj1[:, :, :].rearrange("p h x -> p (h x)"),
                    eb1_s[:, h0 * NT:(h0 + 2) * NT])
                nc.vector.tensor_mul(
                    pj2[0:NT2, :, :].rearrange("p h x -> p (h x)"),
                    pj2[0:NT2, :, :].rearrange("p h x -> p (h x)"),
                    eb2_s[:, h0 * NT:(h0 + 2) * NT])
                return pj1, pj2

            def stage_b(b, hp, pj1, pj2):
                """O^T and denominators, both head-pair col-packed into
                one PSUM tile; one recip + one merged normalize."""
                h0, h1 = 2 * hp, 2 * hp + 1
                ot = mm.tile([128, 512], F32, name="ot", tag="mm")
                nc.tensor.matmul(
                    ot[0:64, 0:NT],
                    vt[b][0][:, h0 * HD:(h0 + 1) * HD],
                    pj1[:, 0, :], start=True, stop=False)
                nc.tensor.matmul(
                    ot[64:128, 0:NT],
                    vt[b][0][:, h1 * HD:(h1 + 1) * HD],
                    pj1[:, 1, :], start=True, stop=False,
                    tile_position=(0, 64))
                nc.tensor.matmul(
                    ot[0:64, 0:NT],
                    vt[b][1][0:NT2, h0 * HD:(h0 + 1) * HD],
                    pj2[0:NT2, 0, :], start=False, stop=True)
                nc.tensor.matmul(
                    ot[64:128, 0:NT],
                    vt[b][1][0:NT2, h1 * HD:(h1 + 1) * HD],
                    pj2[0:NT2, 1, :], start=False, stop=True,
                    tile_position=(0, 64))
                # denominators: ones[128,64] lhsT, heads col-packed into
                # cols 256:256+NT of the same tile (partitions 0:64 = h0,
                # 64:128 = h1); accumulation groups in one PSUM bank must
                # stay sequential (interleaving open groups corrupts them)
                nc.tensor.matmul(
                    ot[0:64, 256:256 + NT], ones64[:, :],
                    pj1[:, 0, :], start=True, stop=False)
                nc.tensor.matmul(
                    ot[64:128, 256:256 + NT], ones64[:, :],
                    pj1[:, 1, :], start=True, stop=False,
                    tile_position=(0, 64))
                nc.tensor.matmul(
                    ot[0:64, 256:256 + NT], ones64[0:NT2, :],
                    pj2[0:NT2, 0, :], start=False, stop=True)
                nc.tensor.matmul(
                    ot[64:128, 256:256 + NT], ones64[0:NT2, :],
                    pj2[0:NT2, 1, :], start=False, stop=True,
                    tile_position=(0, 64))
                rc = rcp.tile([128, NT], F32, name="rc", tag="rc")
                nc.vector.reciprocal_approx_fast(
                    out=rc[:, :], in_=ot[:, 256:256 + NT])
                nc.vector.tensor_mul(
                    op_s[:, hp, b * NT:(b + 1) * NT],
                    ot[:, 0:NT], rc[:, :])

            # ---- projection over a token range (1-2 batches) ----
            def emit_proj(t0, t1):
                w = t1 - t0
                for c in range(KC):
                    acc = mm.tile([128, 512], F32, name="acc_p", tag="mm")
                    for kp in range(KC):
                        nc.tensor.matmul(
                            acc[:, 0:w],
                            pw_s[:, kp, c * 128:(c + 1) * 128],
                            op_s[:, kp, t0:t1],
                            start=(kp == 0),
                            stop=(kp == KC - 1) and not proj_bias_nonzero,
                        )
                    if proj_bias_nonzero:
                        nc.tensor.matmul(
                            acc[:, 0:w],
                            pb_s[0:1, c * 128:(c + 1) * 128],
                            ones_bfr[0:1, 0:w],
                            start=False, stop=True,
                        )
                    obt = obp.tile([128, 2 * NT], BF16, name="obt", tag="ob")
                    if c % 2 == 0:
                        nc.vector.tensor_copy(obt[:, 0:w], acc[:, 0:w])
                    else:
                        nc.scalar.copy(obt[:, 0:w], acc[:, 0:w])
                    nc.sync.dma_start(out[c * 128:(c + 1) * 128, t0:t1],
                                      obt[:, 0:w])

            # ---- batch-major schedule: v[b] -> items -> proj[b] ----
            items = [(b, hp) for b in range(BPC) for hp in range(KC)]
            pend = {}

            def run_stage_b(i):
                b, hp = items[i]
                stage_b(b, hp, *pend.pop(i))
                if hp == KC - 1:
                    # batches 0-5 project in pairs (394-wide rhs halves
                    # per-matmul overhead); 6 and 7 go solo so the last
                    # batch's projection isn't delayed into the tail
                    if b in (1, 3, 5):
                        emit_proj((b - 1) * NT, (b + 1) * NT)
                    elif b >= 6:
                        emit_proj(b * NT, (b + 1) * NT)

            for i, (b, hp) in enumerate(items):
                if hp == 0:
                    emit_v(b)
                pend[i] = stage_a(b, hp)
                if i >= SKEW:
                    run_stage_b(i - SKEW)
            for i in range(len(items) - SKEW, len(items)):
                run_stage_b(i)

    nc.compile()
    return nc


@functools.lru_cache(maxsize=4)
def _built(qkv_bias_nonzero: bool, proj_bias_nonzero: bool):
    return build(qkv_bias_nonzero, proj_bias_nonzero)


def prepare_inputs(x, qkv_w, q_bias, v_bias, rpb_table, proj_w, proj_b, rel_index):
    """Host-side prep: shard + transpose + fold scale + gather bias table."""
    x = np.asarray(x, dtype=np.float32)
    qkv_w = np.asarray(qkv_w, dtype=np.float32)
    q_bias = np.asarray(q_bias, dtype=np.float32)
    v_bias = np.asarray(v_bias, dtype=np.float32)
    rpb_table = np.asarray(rpb_table, dtype=np.float32)
    proj_w = np.asarray(proj_w, dtype=np.float32)
    proj_b = np.asarray(proj_b, dtype=np.float32)
    rel_index = np.asarray(rel_index)

    qw = qkv_w[0:DIM] * np.float32(SCALE)   # exact: SCALE is a power of two
    qkw_h = np.ascontiguousarray(
        np.concatenate([qw, qkv_w[DIM:2 * DIM]], axis=0).T).astype(
        ml_dtypes.bfloat16)                                      # [768, 1536]
    vw_h = np.ascontiguousarray(qkv_w[2 * DIM:3 * DIM].T).astype(
        ml_dtypes.bfloat16)                                      # [768, 768]
    pw_h = np.ascontiguousarray(proj_w.T).astype(ml_dtypes.bfloat16)

    # bias[i, j, h] -> exp -> ebT[h, j, i]
    bias = rpb_table[rel_index]                                  # (197,197,12)
    ebT = np.exp(bias.astype(np.float32)).transpose(2, 1, 0)     # (12, j, i)
    eb1_h = np.ascontiguousarray(
        ebT[:, 0:128, :].transpose(1, 0, 2).reshape(128, H * NT)
    ).astype(ml_dtypes.bfloat16)
    eb2_h = np.ascontiguousarray(
        ebT[:, 128:NT, :].transpose(1, 0, 2).reshape(NT2, H * NT)
    ).astype(ml_dtypes.bfloat16)

    qkv_bias_nonzero = bool(q_bias.any() or v_bias.any())
    proj_bias_nonzero = bool(proj_b.any())

    in_maps = []
    for i in range(NCORES):
        xs = x[i * BPC:(i + 1) * BPC].reshape(TOK, DIM)
        m = {
            "xt": np.ascontiguousarray(xs.T).astype(ml_dtypes.bfloat16),
            "qkw": qkw_h, "vw": vw_h, "pw": pw_h,
            "eb1": eb1_h, "eb2": eb2_h,
        }
        if qkv_bias_nonzero:
            m["qkb"] = np.ascontiguousarray(
                np.concatenate([q_bias * np.float32(SCALE),
                                np.zeros_like(q_bias)])[None, :],
                dtype=np.float32).astype(ml_dtypes.bfloat16)
            m["vb"] = np.ascontiguousarray(
                v_bias[None, :]).astype(ml_dtypes.bfloat16)
        if proj_bias_nonzero:
            m["pb"] = np.ascontiguousarray(
                proj_b[None, :], dtype=np.float32).astype(ml_dtypes.bfloat16)
        in_maps.append(m)
    return in_maps, qkv_bias_nonzero, proj_bias_nonzero


def kernel(x, qkv_w, q_bias, v_bias, rpb_table, proj_w, proj_b, rel_index):
    in_maps, qb_nz, pb_nz = prepare_inputs(
        x, qkv_w, q_bias, v_bias, rpb_table, proj_w, proj_b, rel_index)
    nc = _built(qb_nz, pb_nz)
    res = run_bass_kernel_spmd(nc, in_maps, core_ids=list(range(NCORES)))
    outs = []
    for i in range(NCORES):
        ofm = res.results[i]["out"]                  # [768, 1576]
        outs.append(ofm.T.reshape(BPC, NT, DIM))
    return np.concatenate(outs, axis=0).astype(np.float32)
